# revision 40
# baseline (speedup 1.0000x reference)
"""Trainium2 Bass kernel for AnticipatoryRestaurantGNN (TransformerConv x4 + BN + pool).

Strategy (edge-parallel, dst-sorted, chunked AllGather):
  - Sort edges by dst; partition nodes into 8 contiguous ranges with ~equal
    edge counts. Each core owns its node range and ALL edges pointing into it,
    so segment-softmax and scatter-add are core-local.
  - Per layer, each core computes q/k/v for its own nodes; k/v (bf16) are
    AllGathered in TWO row-chunks (lower/upper half of each core's padded
    node range) so the second AllGather overlaps the first chunk's edge
    sweep (edge phase runs as two sweeps, partial sums merged in node phase).
  - Edge compute per 4-tile (512-edge) window: eps=ea@We and q-row-gather via
    host-baked one-hot matmuls on PE; kv_j = kvg + eps as ONE broadcast DVE
    add; per-head logits via wide mul + reduce; exp on ScalarE; av on DVE;
    scatter-add + denominator accumulate in PSUM via one-hot matmuls.
  - k/v rows are fetched with per-tile indirect DMA. (Batched dma_gather and
    tensor_tensor_reduce both hang this environment's runtime — env-gated
    paths KB_GATHER/KB_TTR exist but default off.) GPSIMD descriptor
    generation (~2us/gather instruction, serialized) is the bottleneck, so
    the per-group node phase (softmax normalize, beta gate, BN partial sums)
    is interleaved into the chunk-1 edge sweep to hide under it; the beta
    x_r.wb term is folded into the skip projection as a 257th matmul column.
  - BatchNorm stats and the final pooled head are AllReduced (tiny).
"""

import math
import os
import sys

sys.path.insert(0, "/opt/trn_rl_repo")

import ml_dtypes
import numpy as np

import concourse.bacc as bacc
import concourse.bass as bass
import concourse.mybir as mybir
import concourse.tile as tile
from concourse import library_config
from concourse.bass_utils import run_bass_kernel_spmd
from concourse.masks import make_identity

BF16 = ml_dtypes.bfloat16

N, E, IN_DIM, EDGE_DIM, HID, L, HEADS, G = 50000, 500000, 64, 16, 256, 4, 4, 64
C = HID // HEADS
NCORES = 8
P = 128
EPS = 1e-5
NCHUNK = 2

F32 = mybir.dt.float32
BF = mybir.dt.bfloat16
I16 = mybir.dt.int16


def _roundup(x, m):
    return (x + m - 1) // m * m


def _wrap16(idx):
    """dma_gather index layout: [16, n//16], idx j -> [j % 16, j // 16]."""
    n = idx.shape[0]
    assert n % 16 == 0
    return np.ascontiguousarray(idx.reshape(n // 16, 16).T)


def plan(edge_index, batch):
    """Host-side layout planning. Returns (meta, per_core_arrays)."""
    src, dst = np.asarray(edge_index[0]), np.asarray(edge_index[1])
    batch = np.asarray(batch)

    order = np.argsort(dst, kind="stable")
    s_src = src[order].astype(np.int64)
    s_dst = dst[order].astype(np.int64)

    deg = np.bincount(dst, minlength=N)
    cum = np.concatenate([[0], np.cumsum(deg)])  # cum[n] = first edge of node n

    # node range split, balanced by edge count, at node boundaries
    ns = [0]
    for i in range(1, NCORES):
        tgt = round(E * i / NCORES)
        ns.append(int(np.searchsorted(cum, tgt, side="left")))
    ns.append(N)
    ns = np.array(ns, dtype=np.int64)
    n_own = np.diff(ns)
    n_pad = _roundup(int(n_own.max()), NCHUNK * P)
    NG = n_pad // P
    NG2 = NG // NCHUNK
    HALF = n_pad // NCHUNK
    BANK = NCORES * HALF
    assert BANK <= 32767

    core_of = np.searchsorted(ns[1:], np.arange(N), side="right")
    loc = np.arange(N) - ns[core_of]
    chunk_of = loc // HALF  # which AG chunk the node's kv row lives in
    bankrow = core_of * HALF + (loc - chunk_of * HALF)

    e_core = core_of[s_dst]  # owning core per sorted edge
    e_chunk = chunk_of[s_src]
    e_bankrow = bankrow[s_src]

    # per (core, group, chunk) edge lists (sorted by src bankrow for locality)
    Tseg = np.zeros((NG, NCHUNK), dtype=np.int64)
    per_core_ed = []
    for c in range(NCORES):
        groups = []
        for g in range(NG):
            lo_node = ns[c] + g * P
            hi_node = min(ns[c] + (g + 1) * P, ns[c + 1])
            if lo_node >= ns[c + 1]:
                eidx = np.arange(0, 0)
            else:
                eidx = np.arange(cum[lo_node], cum[hi_node])
            segs = []
            for r in range(NCHUNK):
                er = eidx[e_chunk[eidx] == r]
                er = er[np.argsort(e_bankrow[er], kind="stable")]
                segs.append(er)
                Tseg[g, r] = max(Tseg[g, r], _roundup(len(er), P) // P)
            groups.append(segs)
        per_core_ed.append(groups)

    TOTE = int(Tseg.sum()) * P  # padded edges per core (same on all cores)

    counts = np.bincount(batch, minlength=G).astype(np.float64)

    meta = dict(ns=ns, n_pad=n_pad, NG=NG, NG2=NG2, HALF=HALF, BANK=BANK,
                Tseg=Tseg, TOTE=TOTE, order=order, counts=counts)

    cores = []
    for c in range(NCORES):
        kv_idx = np.zeros(TOTE, dtype=np.int64)
        a_t2 = np.zeros((P, TOTE), dtype=BF16)   # [dst? no: edge-partition one-hot]
        a_gt2 = np.zeros((P, TOTE), dtype=BF16)
        ea_sel = np.full(TOTE, -1, dtype=np.int64)
        off = 0
        for g in range(NG):
            lo_node = ns[c] + g * P
            for r in range(NCHUNK):
                el = per_core_ed[c][g][r]
                T = int(Tseg[g, r])
                if T == 0:
                    continue
                npad = T * P
                k = len(el)
                kvv = np.full(npad, c * HALF, dtype=np.int64)  # pad: valid row
                kvv[:k] = e_bankrow[el]
                kv_idx[off:off + npad] = kvv
                dr = np.full(npad, -1, dtype=np.int64)
                if k:
                    dr[:k] = s_dst[el] - lo_node
                atb = np.zeros((npad, P), dtype=np.float32)
                valid = dr >= 0
                atb[np.arange(npad)[valid], dr[valid]] = 1.0
                for t in range(T):
                    blk = atb[t * P:(t + 1) * P]  # [128e, 128d]
                    a_t2[:, off + t * P: off + (t + 1) * P] = blk.astype(BF16)
                    a_gt2[:, off + t * P: off + (t + 1) * P] = blk.T.astype(BF16)
                ea_sel[off:off + k] = el
                off += npad
        assert off == TOTE

        kv16 = _wrap16(kv_idx.astype(np.int16))  # [16, TOTE//16]
        TT = TOTE // P
        kv_idx32 = np.ascontiguousarray(kv_idx.reshape(TT, P).T.astype(np.int32))
        kvidx_row = np.ascontiguousarray(kv_idx.astype(np.int32)[None, :])

        nn = int(n_own[c])
        invcnt = np.zeros((n_pad, 1), dtype=np.float32)
        nodes = np.arange(ns[c], ns[c + 1])
        invcnt[:nn, 0] = 1.0 / np.maximum(counts[batch[nodes]], 1.0)
        maskcol = np.zeros((n_pad, 1), dtype=np.float32)
        maskcol[:nn, 0] = 1.0
        p_t = np.zeros((n_pad, G), dtype=np.float32)
        p_t[np.arange(nn), batch[nodes]] = 1.0
        maskrep = np.repeat(maskcol[(NG - 1) * P:NG * P, 0][None, :], P, axis=0)

        cores.append(dict(kv16=kv16, kv_idx32=kv_idx32, kvidx_row=kvidx_row,
                          a_t2=a_t2, a_gt2=a_gt2,
                          ea_sel=ea_sel, invcnt=invcnt, maskcol=maskcol,
                          maskrep=maskrep, p_t=p_t.astype(BF16), nn=nn))
    return meta, cores


def build_inmaps(inputs, meta, cores):
    """Build the per-core in_maps dict for run_bass_kernel_spmd."""
    ns, n_pad = meta["ns"], meta["n_pad"]
    order = meta["order"]
    ea_sorted = np.asarray(inputs["edge_attr"])[order]  # [E, 16] in dst-sorted order

    def f32(x):
        return np.ascontiguousarray(np.asarray(x, dtype=np.float32))

    def bf(x):
        return np.ascontiguousarray(np.asarray(x, dtype=np.float32).astype(BF16))

    nf = np.asarray(inputs["node_features"])
    Wbeta = np.asarray(inputs["Wbeta"])  # [L, 768, 1]
    head_W = np.asarray(inputs["head_W"])  # [512, 1]

    shared = {}
    shared["proj_W"] = bf(inputs["proj_W"])                      # [64,256]
    shared["proj_b"] = bf(inputs["proj_b"][None, :])             # [1,256]
    for l in range(L):
        shared[f"Wq{l}"] = bf(inputs["Wq"][l])
        shared[f"Wk{l}"] = bf(inputs["Wk"][l])
        shared[f"Wv{l}"] = bf(inputs["Wv"][l])
        shared[f"Wsk{l}"] = bf(inputs["Wskip"][l])
        shared[f"bq{l}"] = bf(inputs["bq"][l][None, :])
        shared[f"bk{l}"] = bf((inputs["bk"][l] + inputs["be"][l])[None, :])
        shared[f"bv{l}"] = bf((inputs["bv"][l] + inputs["be"][l])[None, :])
        shared[f"bsk{l}"] = bf(inputs["bskip"][l][None, :])
        shared[f"We{l}"] = bf(inputs["We"][l])                   # [16,256]
        w1, w2, w3 = Wbeta[l, :HID, 0], Wbeta[l, HID:2 * HID, 0], Wbeta[l, 2 * HID:, 0]
        shared[f"wa{l}"] = bf(np.repeat((w1 + w3)[None, :], P, axis=0))   # [128,256]
        shared[f"wb{l}"] = bf(np.repeat((w2 - w3)[None, :], P, axis=0))
        wbv = (w2 - w3).astype(np.float64)
        wsk64 = np.asarray(inputs["Wskip"][l], dtype=np.float64)
        wsbv = wsk64 @ wbv                                   # [256]
        shared[f"wsb{l}"] = bf(np.stack([wsbv[:P], wsbv[P:]], axis=1))  # [128,2]
        bwc = float(np.asarray(inputs["bskip"][l], dtype=np.float64) @ wbv)
        shared[f"bwr{l}"] = f32(np.full((P, 1), bwc))
        shared[f"gam{l}"] = f32(inputs["bn_gamma"][l][None, :])
        shared[f"bet{l}"] = f32(inputs["bn_beta"][l][None, :])
    shared["h1r"] = bf(np.repeat(head_W[:HID, 0][None, :], P, axis=0))
    shared["h2r"] = bf(np.repeat(head_W[HID:, 0][None, :], P, axis=0))

    in_maps = []
    for c in range(NCORES):
        cc = cores[c]
        m = dict(shared)
        nn = cc["nn"]
        nft = np.zeros((IN_DIM, n_pad), dtype=np.float32)
        nft[:, :nn] = nf[ns[c]:ns[c + 1]].T
        m["nf_t"] = nft.astype(BF16)
        ea = np.zeros((meta["TOTE"], EDGE_DIM), dtype=np.float32)
        sel = cc["ea_sel"]
        ea[sel >= 0] = ea_sorted[sel[sel >= 0]]
        m["ea_t"] = np.ascontiguousarray(ea.T.astype(BF16))     # [16, TOTE] bf16
        m["a_t2"] = cc["a_t2"]                                  # [128, TOTE] bf16
        m["a_gt2"] = cc["a_gt2"]
        m["kv16"] = cc["kv16"]                                  # [16, TOTE//16] int16
        m["kv_idx32"] = cc["kv_idx32"]                          # [128, TOTE//128] int32
        m["kvidx_row"] = cc["kvidx_row"]                        # [1, TOTE] int32
        m["invcnt"] = cc["invcnt"]
        m["maskcol"] = cc["maskcol"]
        m["maskrep"] = cc["maskrep"].astype(BF16)
        m["p_t"] = cc["p_t"]
        in_maps.append(m)
    return in_maps


def build(nc, meta, head_b):
    n_pad, NG, NG2, HALF, BANK, Tseg, TOTE = (
        meta["n_pad"], meta["NG"], meta["NG2"], meta["HALF"], meta["BANK"],
        meta["Tseg"], meta["TOTE"])

    # ---- dram parameters ----
    dp = lambda name, shape, dt: nc.declare_dram_parameter(name, list(shape), dt, isOutput=False)
    nf_t = dp("nf_t", (IN_DIM, n_pad), BF)
    ea_t = dp("ea_t", (EDGE_DIM, TOTE), BF)
    a_t2 = dp("a_t2", (P, TOTE), BF)
    a_gt2 = dp("a_gt2", (P, TOTE), BF)
    kv16 = dp("kv16", (16, TOTE // 16), I16)
    kv_idx32 = dp("kv_idx32", (P, TOTE // P), mybir.dt.int32)
    kvidx_row = dp("kvidx_row", (1, TOTE), mybir.dt.int32)
    invcnt = dp("invcnt", (n_pad, 1), F32)
    maskcol = dp("maskcol", (n_pad, 1), F32)
    maskrep = dp("maskrep", (P, P), BF)
    p_t = dp("p_t", (n_pad, G), BF)
    proj_W = dp("proj_W", (IN_DIM, HID), BF)
    proj_b = dp("proj_b", (1, HID), BF)
    Wq, Wk, Wv, Wsk, bq, bk, bv, bsk, We, wa, wb, gam, bet = ({} for _ in range(13))
    for l in range(L):
        Wq[l] = dp(f"Wq{l}", (HID, HID), BF)
        Wk[l] = dp(f"Wk{l}", (HID, HID), BF)
        Wv[l] = dp(f"Wv{l}", (HID, HID), BF)
        Wsk[l] = dp(f"Wsk{l}", (HID, HID), BF)
        bq[l] = dp(f"bq{l}", (1, HID), BF)
        bk[l] = dp(f"bk{l}", (1, HID), BF)
        bv[l] = dp(f"bv{l}", (1, HID), BF)
        bsk[l] = dp(f"bsk{l}", (1, HID), BF)
        We[l] = dp(f"We{l}", (EDGE_DIM, HID), BF)
        wa[l] = dp(f"wa{l}", (P, HID), BF)
        wb[l] = dp(f"wb{l}", (P, HID), BF)
        wa[l, "wsb"] = dp(f"wsb{l}", (P, 2), BF)
        wa[l, "bwr"] = dp(f"bwr{l}", (P, 1), F32)
        gam[l] = dp(f"gam{l}", (1, HID), F32)
        bet[l] = dp(f"bet{l}", (1, HID), F32)
    h1r = dp("h1r", (P, HID), BF)
    h2r = dp("h2r", (P, HID), BF)
    out_ext = nc.declare_dram_parameter("out", [G, 1], F32, isOutput=True)

    cg = [list(range(NCORES))]

    with tile.TileContext(nc) as tc:
        from contextlib import ExitStack
        ctx = ExitStack()

        # ---- DRAM tiles (pool.tile keeps addr_space; tc.tile drops it) ----
        dpool = ctx.enter_context(tc.tile_pool(name="dram", bufs=1, space="DRAM"))
        shared_kw = ({"addr_space": "Shared"}
                     if os.environ.get("KB_SHARED", "0") == "1" else {})
        kv_slab = [[dpool.tile([HALF, 2 * HID], BF, name=f"kv_slab{l}_{r}")
                    for r in range(NCHUNK)] for l in range(L)]
        q_slab = dpool.tile([n_pad, HID], BF, name="q_slab")
        kv_bank = [[dpool.tile([BANK, 2 * HID], BF, name=f"kv_bank{l}_{r}",
                               **shared_kw) for r in range(NCHUNK)]
                   for l in range(L)]
        st_in = dpool.tile([2, HID], F32, name="st_in")
        st_out = [dpool.tile([2, HID], F32, name=f"st_out{l}", **shared_kw)
                  for l in range(L)]
        pool_in = dpool.tile([G, 1], F32, name="pool_in")
        pool_out = dpool.tile([G, 1], F32, name="pool_out", **shared_kw)

        # ---- persistent SBUF ----
        res = ctx.enter_context(tc.tile_pool(name="res", bufs=1))
        x_T = [res.tile([P, n_pad], BF, tag=f"xT{h}", name=f"xT{h}") for h in range(2)]
        xnew = res.tile([P, NG * HID], BF, tag="xnew", name="xnew")
        out_sbA = res.tile([P, NG * HID], BF, tag="outsbA", name="outsbA")
        out_sbB = res.tile([P, NG * HID], BF, tag="outsbB", name="outsbB")
        scr_res = res.tile([P, HID], F32, tag="scrres", name="scrres")
        scr_bf = res.tile([P, HID], BF, tag="scrbf", name="scrbf")
        den_sbA = res.tile([P, NG * HEADS], F32, tag="densbA", name="densbA")
        den_sbB = res.tile([P, NG * HEADS], F32, tag="densbB", name="densbB")
        ident = res.tile([P, P], BF, tag="ident", name="ident")
        make_identity(nc, ident[:])
        ones_row = res.tile([1, P], BF, tag="onesr", name="onesr")
        nc.vector.memset(ones_row[:], 1.0)
        ones_col = res.tile([P, 1], BF, tag="onesc", name="onesc")
        nc.vector.memset(ones_col[:], 1.0)
        one1 = res.tile([1, 1], BF, tag="one1", name="one1")
        nc.vector.memset(one1[:], 1.0)
        wa_sb = res.tile([P, HID], BF, tag="wasb", name="wasb")
        wb_sb = res.tile([P, HID], BF, tag="wbsb", name="wbsb")
        wsb_sb = res.tile([P, 2], BF, tag="wsbsb", name="wsbsb")
        bw_sb = res.tile([P, 1], F32, tag="bwsb", name="bwsb")
        aT_sb = res.tile([P, 2], F32, tag="aTsb", name="aTsb")   # col h = scale for half h
        cT_sb = res.tile([P, 2], F32, tag="cTsb", name="cTsb")
        WW = res.tile([P, 8 * HID], BF, tag="WW", name="WW")  # q0 q1 k0 k1 v0 v1 s0 s1
        We_sb = res.tile([EDGE_DIM, HID], BF, tag="Wesb", name="Wesb")
        bias_sb = res.tile([1, 4 * HID], BF, tag="biassb", name="biassb")  # bq bk bv bsk
        KB_GATHER = int(os.environ.get("KB_GATHER", "0"))
        if KB_GATHER == 3:
            gsem = nc.alloc_semaphore(name="kvgsem")
            gcount = [0]
        KB_TTR = os.environ.get("KB_TTR", "0") == "1"
        if KB_GATHER != 1:
            kvidx32_sb = res.tile([P, TOTE // P], mybir.dt.int32, tag="kvidx32",
                                  name="kvidx32")
            nc.sync.dma_start(out=kvidx32_sb[:], in_=kv_idx32[:])

        h1r_sb = res.tile([P, HID], BF, tag="h1rsb", name="h1rsb")
        nc.sync.dma_start(out=h1r_sb[:], in_=h1r[:])
        h2r_sb = res.tile([P, HID], BF, tag="h2rsb", name="h2rsb")
        nc.sync.dma_start(out=h2r_sb[:], in_=h2r[:])

        for h in range(2):
            nc.vector.memset(x_T[h][:], 0.01)
        nc.vector.memset(xnew[:], 0.01)
        nc.vector.memset(aT_sb[:], 1.0)
        nc.vector.memset(cT_sb[:], 0.0)

        wpool = ctx.enter_context(tc.tile_pool(name="wtmp", bufs=1))
        psum = ctx.enter_context(tc.tile_pool(name="ps", bufs=2, space="PSUM"))
        sb = ctx.enter_context(tc.tile_pool(name="sb", bufs=2))

        def scope_in(name):
            sid, _ = nc.enter_named_scope(name, False)
            return (name, sid)

        def scope_out(tok):
            nc.leave_named_scope(tok[0], tok[1], False)

        def load_layer_weights(l):
            for i, W in enumerate((Wq[l], Wk[l], Wv[l], Wsk[l])):
                for h in range(2):
                    nc.sync.dma_start(out=WW[:, (2 * i + h) * HID:(2 * i + h + 1) * HID],
                                      in_=W[h * P:(h + 1) * P, :])
            nc.sync.dma_start(out=We_sb[:], in_=We[l][:])
            for i, b in enumerate((bq[l], bk[l], bv[l], bsk[l])):
                nc.sync.dma_start(out=bias_sb[:, i * HID:(i + 1) * HID], in_=b[:])
            nc.sync.dma_start(out=wa_sb[:], in_=wa[l][:])
            nc.sync.dma_start(out=wb_sb[:], in_=wb[l][:])
            nc.sync.dma_start(out=wsb_sb[:], in_=wa[l, "wsb"][:])
            nc.sync.dma_start(out=bw_sb[:], in_=wa[l, "bwr"][:])

        def proj_into(t, widx, dst_slab):
            """project x_T node-tile t with weight widx (0..3 -> q,k,v,skip).
            Returns (o, ps): sbuf bf16 [P, HID] tile and the psum (skip proj
            also computes column HID = x . (Wskip @ wb) + bskip . wb, the
            beta-gate x_r term). DMAs to dst_slab if given."""
            wcols = HID + 1 if widx == 3 else HID
            ps = psum.tile([P, HID + 1], F32, tag="mm", name="projps")
            nc.tensor.matmul(ps[:, :HID], x_T[0][:, t * P:(t + 1) * P],
                             WW[:, (2 * widx) * HID:(2 * widx + 1) * HID],
                             start=True, stop=False)
            nc.tensor.matmul(ps[:, :HID], x_T[1][:, t * P:(t + 1) * P],
                             WW[:, (2 * widx + 1) * HID:(2 * widx + 2) * HID],
                             start=False, stop=False)
            nc.tensor.matmul(ps[:, :HID], ones_row[:],
                             bias_sb[:, widx * HID:(widx + 1) * HID],
                             start=False, stop=True)
            if widx == 3:
                nc.tensor.matmul(ps[:, HID:HID + 1], x_T[0][:, t * P:(t + 1) * P],
                                 wsb_sb[:, 0:1], start=True, stop=False)
                nc.tensor.matmul(ps[:, HID:HID + 1], x_T[1][:, t * P:(t + 1) * P],
                                 wsb_sb[:, 1:2], start=False, stop=True)
            o = sb.tile([P, HID], BF, tag="projo", name="projo")
            nc.scalar.activation(o[:], ps[:, :HID], mybir.ActivationFunctionType.Copy)
            if dst_slab is not None:
                slab, row0, c0 = dst_slab
                nc.sync.dma_start(out=slab[row0:row0 + P, c0:c0 + HID], in_=o[:])
            return o, ps

        _agn = [0]

        _agn = [0]

        def emit_ag(l, r):
            sid, _ = nc.enter_named_scope(f"ag{_agn[0]}", False)
            nc.gpsimd.collective_compute("AllGather", mybir.AluOpType.bypass,
                                         replica_groups=cg,
                                         ins=[kv_slab[l][r][:].opt()],
                                         outs=[kv_bank[l][r][:].opt()])
            nc.leave_named_scope(f"ag{_agn[0]}", sid, False)
            _agn[0] += 1

        def qkv_phase(l):
            """q/k/v slabs from x_T; AllGather chunk 0 now, chunk 1 is
            triggered later, interleaved into the chunk-0 edge sweep."""
            for r in range(NCHUNK):
                for t2 in range(NG2):
                    t = r * NG2 + t2
                    proj_into(t, 0, (q_slab, t * P, 0))
                    proj_into(t, 1, (kv_slab[l][r], t2 * P, 0))
                    proj_into(t, 2, (kv_slab[l][r], t2 * P, HID))
                if r == 0:
                    emit_ag(l, 0)

        # edge-tile offsets: seg (g, r) starts at toff[g][r] (in 128-tiles)
        toff = np.zeros((NG, NCHUNK), dtype=np.int64)
        acc = 0
        for g in range(NG):
            for r in range(NCHUNK):
                toff[g, r] = acc
                acc += int(Tseg[g, r])

        KB_SKIPEDGE = os.environ.get("KB_SKIPEDGE", "") == "1"

        def edge_seg(l, g, r, out_sb, den_sb):
            """process edge segment (group g, chunk r) gathering from kv_bank[r]."""
            T = 0 if KB_SKIPEDGE else int(Tseg[g, r])
            og = out_sb[:, g * HID:(g + 1) * HID]
            dg = den_sb[:, g * HEADS:(g + 1) * HEADS]
            if T == 0:
                nc.vector.memset(og, 0.0)
                nc.vector.memset(dg, 0.0)
                return
            t0 = int(toff[g, r])
            GW = 4  # tiles per DVE window
            TMAX = int(Tseg.max())
            kvgs = []
            if KB_GATHER == 4:
                rid = sb.tile([1, TMAX * P], mybir.dt.int32, tag="kvrow",
                              name="kvrow", bufs=2)
                nc.sync.dma_start(out=rid[:, :T * P],
                                  in_=kvidx_row[:, t0 * P:(t0 + T) * P])
            if KB_GATHER == 3:
                kvg = sb.tile([P, TMAX * 2 * HID], BF, tag="kvg3", name="kvg3",
                              bufs=2)
                idx = sb.tile([16, TMAX * 8], I16, tag="kvidx", name="kvidx",
                              bufs=2)
                nc.sync.dma_start(out=idx[:, :T * 8],
                                  in_=kv16[:, t0 * 8:(t0 + T) * 8])
                nc.gpsimd.dma_gather(
                    kvg[:, :T * 2 * HID].rearrange("p (t c) -> p t c", c=2 * HID),
                    kv_bank[l][r][:],
                    idx[:, :T * 8],
                    T * P, T * P, 2 * HID).then_inc(gsem, 16)
                gcount[0] += 1
                nc.vector.wait_ge(gsem, 16 * gcount[0])
                kvgs = [kvg] * ((T + GW - 1) // GW)
            for w0 in ([] if KB_GATHER == 3 else list(range(0, T, GW))):
                tw4 = min(GW, T - w0)
                kvg = sb.tile([P, GW * 2 * HID], BF, tag="kvg", name="kvg", bufs=2)
                if KB_GATHER == 1:
                    idx = sb.tile([16, GW * 8], I16, tag="kvidx", name="kvidx", bufs=2)
                    nc.sync.dma_start(out=idx[:, :tw4 * 8],
                                      in_=kv16[:, (t0 + w0) * 8:(t0 + w0 + tw4) * 8])
                    nc.gpsimd.dma_gather(
                        kvg[:, :tw4 * 2 * HID].rearrange("p (t c) -> p t c", c=2 * HID),
                        kv_bank[l][r][:],
                        idx[:, :tw4 * 8],
                        tw4 * P, tw4 * P, 2 * HID)
                elif KB_GATHER == 2:
                    nc.gpsimd.indirect_dma_start(
                        out=kvg[:, :tw4 * 2 * HID].rearrange("p (t c) -> p t c",
                                                             c=2 * HID),
                        out_offset=None, in_=kv_bank[l][r][:],
                        in_offset=bass.IndirectOffsetOnAxis(
                            ap=kvidx32_sb[:, t0 + w0:t0 + w0 + tw4], axis=0))
                elif KB_GATHER == 4:
                    for ti in range(tw4):
                        s0 = (w0 + ti) * P
                        nc.gpsimd.indirect_dma_start(
                            out=kvg[:, ti * 2 * HID:(ti + 1) * 2 * HID],
                            out_offset=None, in_=kv_bank[l][r][:],
                            in_offset=bass.IndirectOffsetOnAxis(
                                ap=rid[:, s0:s0 + P], axis=0))
                else:
                    for ti in range(tw4):
                        gt2 = t0 + w0 + ti
                        nc.gpsimd.indirect_dma_start(
                            out=kvg[:, ti * 2 * HID:(ti + 1) * 2 * HID],
                            out_offset=None, in_=kv_bank[l][r][:],
                            in_offset=bass.IndirectOffsetOnAxis(
                                ap=kvidx32_sb[:, gt2:gt2 + 1], axis=0))
                kvgs.append(kvg)
            qg_sb = sb.tile([P, HID], BF, tag="qgrp", name="qgrp", bufs=2)
            nc.sync.dma_start(out=qg_sb[:], in_=q_slab[g * P:(g + 1) * P, :])
            ps_out = psum.tile([P, HID], F32, tag="acc", name="accps", bufs=1)
            ps_den = psum.tile([P, 8], F32, tag="den", name="denps", bufs=1)
            for w0 in range(0, T, GW):
                tw4 = min(GW, T - w0)
                gt = t0 + w0
                kvg = kvgs[w0 // GW]
                kvo0 = w0 * 2 * HID if KB_GATHER == 3 else 0
                ea_sb = sb.tile([EDGE_DIM, GW * P], BF, tag="ea", name="ea")
                nc.sync.dma_start(out=ea_sb[:, :tw4 * P],
                                  in_=ea_t[:, gt * P:(gt + tw4) * P])
                at_sb = sb.tile([P, GW * P], BF, tag="at", name="at")
                nc.sync.dma_start(out=at_sb[:, :tw4 * P],
                                  in_=a_t2[:, gt * P:(gt + tw4) * P])
                agt_sb = sb.tile([P, GW * P], BF, tag="agt", name="agt")
                nc.sync.dma_start(out=agt_sb[:, :tw4 * P],
                                  in_=a_gt2[:, gt * P:(gt + tw4) * P])
                eps_sb = sb.tile([P, GW * HID], BF, tag="epssb", name="epssb")
                qi_sb = sb.tile([P, GW * HID], BF, tag="qisb", name="qisb")
                for hh in range(0, tw4, 2):
                    hw = min(2, tw4 - hh)
                    eps_ps = psum.tile([P, 2 * HID], F32, tag="mm", name="epsps")
                    qps = psum.tile([P, 2 * HID], F32, tag="mm2", name="qps")
                    for j in range(hw):
                        t = hh + j
                        nc.tensor.matmul(eps_ps[:, j * HID:(j + 1) * HID],
                                         ea_sb[:, t * P:(t + 1) * P], We_sb[:],
                                         start=True, stop=True)
                        nc.tensor.matmul(qps[:, j * HID:(j + 1) * HID],
                                         agt_sb[:, t * P:(t + 1) * P], qg_sb[:],
                                         start=True, stop=True)
                    nc.scalar.activation(eps_sb[:, hh * HID:(hh + hw) * HID],
                                         eps_ps[:, :hw * HID],
                                         mybir.ActivationFunctionType.Copy)
                    nc.scalar.activation(qi_sb[:, hh * HID:(hh + hw) * HID],
                                         qps[:, :hw * HID],
                                         mybir.ActivationFunctionType.Copy)
                kvj = sb.tile([P, GW * 2 * HID], BF, tag="kvj", name="kvj")
                nc.vector.tensor_tensor(
                    out=kvj[:, :tw4 * 2 * HID
                            ].rearrange("p (t kv c) -> p t kv c", kv=2, c=HID),
                    in0=kvg[:, kvo0:kvo0 + tw4 * 2 * HID
                            ].rearrange("p (t kv c) -> p t kv c", kv=2, c=HID),
                    in1=eps_sb[:, :tw4 * HID
                               ].rearrange("p (t o c) -> p t o c", o=1, c=HID
                                           ).to_broadcast([P, tw4, 2, HID]),
                    op=mybir.AluOpType.add)
                kvjv = kvj[:, :tw4 * 2 * HID].rearrange("p (t k) -> p t k",
                                                        k=2 * HID)
                prod = sb.tile([P, GW * HID], BF, tag="prod", name="prod")
                nc.vector.tensor_tensor(
                    out=prod[:, :tw4 * HID].rearrange("p (t c) -> p t c", c=HID),
                    in0=qi_sb[:, :tw4 * HID].rearrange("p (t c) -> p t c", c=HID),
                    in1=kvjv[:, :, :HID],
                    op=mybir.AluOpType.mult)
                logit = sb.tile([P, GW * HEADS], F32, tag="logit", name="logit")
                nc.vector.reduce_sum(
                    logit[:, :tw4 * HEADS],
                    prod[:, :tw4 * HID].rearrange("p (th c) -> p th c", c=C),
                    axis=mybir.AxisListType.X)
                alpha = sb.tile([P, GW * HEADS], BF, tag="alpha", name="alpha")
                nc.scalar.activation(alpha[:, :tw4 * HEADS], logit[:, :tw4 * HEADS],
                                     mybir.ActivationFunctionType.Exp,
                                     scale=1.0 / math.sqrt(C))
                av = sb.tile([P, GW * HID], BF, tag="av", name="av")
                nc.vector.tensor_tensor(
                    out=av[:, :tw4 * HID
                           ].rearrange("p (t h c) -> p t h c", h=HEADS, c=C),
                    in0=kvjv[:, :, HID:].rearrange("p t (h c) -> p t h c", c=C),
                    in1=alpha[:, :tw4 * HEADS
                              ].rearrange("p (t h o) -> p t h o", h=HEADS, o=1
                                          ).to_broadcast([P, tw4, HEADS, C]),
                    op=mybir.AluOpType.mult)
                first = w0 == 0
                for t in range(tw4):
                    last = w0 + t == T - 1
                    nc.tensor.matmul(ps_out[:], at_sb[:, t * P:(t + 1) * P],
                                     av[:, t * HID:(t + 1) * HID],
                                     start=(first and t == 0), stop=last)
                    nc.tensor.matmul(ps_den[:, :HEADS], at_sb[:, t * P:(t + 1) * P],
                                     alpha[:, t * HEADS:(t + 1) * HEADS],
                                     start=(first and t == 0), stop=last)
            nc.scalar.activation(og, ps_out[:], mybir.ActivationFunctionType.Copy)
            nc.scalar.activation(dg, ps_den[:, :HEADS],
                                 mybir.ActivationFunctionType.Copy)

        # ================= init: x0 = nf @ proj_W + proj_b =================
        tok = scope_in("x0qkv")
        pw_sb = wpool.tile([IN_DIM, HID], BF, tag="pw", name="pw")
        nc.sync.dma_start(out=pw_sb[:], in_=proj_W[:])
        pb_sb = wpool.tile([1, HID], BF, tag="pb", name="pb")
        nc.sync.dma_start(out=pb_sb[:], in_=proj_b[:])
        load_layer_weights(0)
        for r in range(NCHUNK):
            for t2 in range(NG2):
                t = r * NG2 + t2
                nfs = sb.tile([IN_DIM, P], BF, tag="nfs", name="nfs")
                nc.sync.dma_start(out=nfs[:], in_=nf_t[:, t * P:(t + 1) * P])
                ps = psum.tile([P, HID], F32, tag="mm", name="x0ps")
                nc.tensor.matmul(ps[:], nfs[:], pw_sb[:],
                                 start=True, stop=False)
                nc.tensor.matmul(ps[:], ones_row[:], pb_sb[:], start=False, stop=True)
                xr0 = sb.tile([P, HID], BF, tag="xrow0", name="xrow0")
                nc.scalar.activation(xr0[:], ps[:], mybir.ActivationFunctionType.Copy)
                for h in range(2):
                    tp = psum.tile([P, P], BF, tag="mm", name="x0tp")
                    nc.tensor.transpose(tp[:], xr0[:, h * P:(h + 1) * P], ident[:])
                    nc.vector.tensor_copy(x_T[h][:, t * P:(t + 1) * P], tp[:])
                proj_into(t, 0, (q_slab, t * P, 0))
                proj_into(t, 1, (kv_slab[0][r], t2 * P, 0))
                proj_into(t, 2, (kv_slab[0][r], t2 * P, HID))
            if r == 0:
                emit_ag(0, 0)
        scope_out(tok)

        # ================= layers =================
        for l in range(L):
            # ---- edge phase: all chunk-0 segments, then all chunk-1 ----
            tok = scope_in(f"edge{l}")
            ps_sum = psum.tile([1, HID], F32, tag="stat1", name="sumps", bufs=1)
            ps_sq = psum.tile([1, HID], F32, tag="stat2", name="sqps", bufs=1)

            def nodeA_group(t):
                den = sb.tile([P, HEADS], F32, tag="den2", name="den2")
                nc.vector.tensor_add(den[:], den_sbA[:, t * HEADS:(t + 1) * HEADS],
                                     den_sbB[:, t * HEADS:(t + 1) * HEADS])
                dmx = sb.tile([P, HEADS], F32, tag="dmx", name="dmx")
                nc.vector.tensor_scalar_max(dmx[:], den[:], 1e-30)
                rden = sb.tile([P, HEADS], F32, tag="rden", name="rden")
                nc.vector.reciprocal(rden[:], dmx[:])
                outm = sb.tile([P, HID], BF, tag="outm", name="outm")
                nc.vector.tensor_add(outm[:], out_sbA[:, t * HID:(t + 1) * HID],
                                     out_sbB[:, t * HID:(t + 1) * HID])
                outn = sb.tile([P, HID], BF, tag="outn", name="outn")
                nc.vector.tensor_tensor(
                    out=outn[:].rearrange("p (h c) -> p h c", c=C),
                    in0=outm[:].rearrange("p (h c) -> p h c", c=C),
                    in1=rden[:].rearrange("p (h o) -> p h o", o=1
                                          ).to_broadcast([P, HEADS, C]),
                    op=mybir.AluOpType.mult)
                xr, xr_ps = proj_into(t, 3, None)  # x_r = x@Wskip + bskip
                bl = sb.tile([P, 2], F32, tag="bl", name="bl")
                nc.vector.tensor_mul(scr_res[:], outn[:], wa_sb[:])
                nc.vector.reduce_sum(bl[:, 0:1], scr_res[:], axis=mybir.AxisListType.X)
                blsum = sb.tile([P, 1], F32, tag="blsum", name="blsum")
                nc.vector.tensor_add(blsum[:], bl[:, 0:1], xr_ps[:, HID:HID + 1])
                bsig = sb.tile([P, 1], F32, tag="bsig", name="bsig")
                nc.scalar.activation(bsig[:], blsum[:],
                                     mybir.ActivationFunctionType.Sigmoid,
                                     bias=bw_sb[:, 0:1])
                diff = sb.tile([P, HID], BF, tag="diff", name="diff")
                nc.vector.tensor_sub(diff[:], xr[:], outn[:])
                bd = sb.tile([P, HID], BF, tag="bd", name="bd")
                nc.vector.tensor_scalar_mul(bd[:], diff[:], bsig[:, 0:1])
                xn = xnew[:, t * HID:(t + 1) * HID]
                nc.vector.tensor_add(xn, outn[:], bd[:])
                mc = sb.tile([P, 1], F32, tag="mc", name="mc")
                nc.sync.dma_start(out=mc[:], in_=maskcol[t * P:(t + 1) * P, :])
                nc.vector.tensor_scalar_mul(xn, xn, mc[:, 0:1])
                x2 = sb.tile([P, HID], BF, tag="x2", name="x2")
                nc.vector.tensor_mul(x2[:], xn, xn)
                nc.tensor.matmul(ps_sum[:], ones_col[:], xn,
                                 start=(t == 0), stop=(t == NG - 1))
                nc.tensor.matmul(ps_sq[:], ones_col[:], x2[:],
                                 start=(t == 0), stop=(t == NG - 1))

            for g in range(NG):
                edge_seg(l, g, 0, out_sbA, den_sbA)
                if g == 1:
                    emit_ag(l, 1)
            if NG <= 1:
                emit_ag(l, 1)
            for g in range(NG):
                edge_seg(l, g, 1, out_sbB, den_sbB)
                nodeA_group(g)
            scope_out(tok)

            # ---- BN stats all-reduce + finalize ----
            tok = scope_in(f"stats{l}")
            st_a = sb.tile([1, HID], F32, tag="stsa", name="stsa")
            nc.vector.tensor_copy(st_a[:], ps_sum[:])
            st_b = sb.tile([1, HID], F32, tag="stsb", name="stsb")
            nc.vector.tensor_copy(st_b[:], ps_sq[:])
            nc.sync.dma_start(out=st_in[0:1, :], in_=st_a[:])
            nc.sync.dma_start(out=st_in[1:2, :], in_=st_b[:])
            nc.gpsimd.collective_compute("AllReduce", mybir.AluOpType.add,
                                         replica_groups=cg,
                                         ins=[st_in[:].opt()], outs=[st_out[l][:].opt()])
            str_a = sb.tile([1, HID], F32, tag="stra", name="stra")
            nc.sync.dma_start(out=str_a[:], in_=st_out[l][0:1, :])
            str_b = sb.tile([1, HID], F32, tag="strb", name="strb")
            nc.sync.dma_start(out=str_b[:], in_=st_out[l][1:2, :])
            mean = sb.tile([1, HID], F32, tag="mean", name="mean")
            nc.vector.tensor_scalar_mul(mean[:], str_a[:], 1.0 / N)
            var = sb.tile([1, HID], F32, tag="var", name="var")
            nc.vector.tensor_scalar_mul(var[:], str_b[:], 1.0 / N)
            msq = sb.tile([1, HID], F32, tag="msq", name="msq")
            nc.vector.tensor_mul(msq[:], mean[:], mean[:])
            nc.vector.tensor_sub(var[:], var[:], msq[:])
            nc.vector.tensor_scalar_add(var[:], var[:], EPS)
            sd = sb.tile([1, HID], F32, tag="sd", name="sd")
            nc.scalar.activation(sd[:], var[:], mybir.ActivationFunctionType.Sqrt)
            rstd = sb.tile([1, HID], F32, tag="rstd", name="rstd")
            nc.vector.reciprocal(rstd[:], sd[:])
            gam_sb = sb.tile([1, HID], F32, tag="gamsb", name="gamsb")
            nc.sync.dma_start(out=gam_sb[:], in_=gam[l][:])
            bet_sb = sb.tile([1, HID], F32, tag="betsb", name="betsb")
            nc.sync.dma_start(out=bet_sb[:], in_=bet[l][:])
            aa = sb.tile([1, HID], BF, tag="aa", name="aa")
            nc.vector.tensor_mul(aa[:], gam_sb[:], rstd[:])
            ac = sb.tile([1, HID], F32, tag="acs", name="acs")
            nc.vector.tensor_mul(ac[:], mean[:], aa[:])
            ccs = sb.tile([1, HID], BF, tag="ccs", name="ccs")
            nc.vector.tensor_sub(ccs[:], bet_sb[:], ac[:])
            for h in range(2):
                tpa = psum.tile([P, 2], F32, tag="mm", name="tpa")
                nc.tensor.matmul(tpa[:, 0:1], aa[0:1, h * P:(h + 1) * P], one1[:],
                                 start=True, stop=True)
                nc.tensor.matmul(tpa[:, 1:2], ccs[0:1, h * P:(h + 1) * P], one1[:],
                                 start=True, stop=True)
                nc.vector.tensor_copy(aT_sb[:, h:h + 1], tpa[:, 0:1])
                nc.vector.tensor_copy(cT_sb[:, h:h + 1], tpa[:, 1:2])
            scope_out(tok)

            # ---- node phase B: transpose + BN apply + LeakyReLU -> x_T ----
            tok = scope_in(f"nodeB{l}" if l < L - 1 else "pool")
            if l < L - 1:
                mrep = sb.tile([P, P], BF, tag="mrep", name="mrep")
                nc.sync.dma_start(out=mrep[:], in_=maskrep[:])
                load_layer_weights(l + 1)
                # fused BN-apply + next-layer projection per tile, so the
                # chunk-0 AllGather triggers as early as possible
                for r in range(NCHUNK):
                    for t2 in range(NG2):
                        t = r * NG2 + t2
                        for h in range(2):
                            tp = psum.tile([P, P], BF, tag="mm", name="xtp")
                            nc.tensor.transpose(
                                tp[:], xnew[:, t * HID + h * P:t * HID + (h + 1) * P],
                                ident[:])
                            dst = x_T[h][:, t * P:(t + 1) * P]
                            nc.scalar.activation(dst, tp[:],
                                                 mybir.ActivationFunctionType.Lrelu,
                                                 bias=cT_sb[:, h:h + 1],
                                                 scale=aT_sb[:, h:h + 1],
                                                 alpha=0.1)
                            if t == NG - 1:
                                nc.vector.tensor_mul(dst, dst, mrep[:])
                        proj_into(t, 0, (q_slab, t * P, 0))
                        proj_into(t, 1, (kv_slab[l + 1][r], t2 * P, 0))
                        proj_into(t, 2, (kv_slab[l + 1][r], t2 * P, HID))
                    if r == 0:
                        emit_ag(l + 1, 0)
            else:
                # ---- pooling + head (x of last layer = BN+lrelu of xnew) ----
                arep_ps = psum.tile([P, HID], F32, tag="mm", name="arep")
                nc.tensor.matmul(arep_ps[:], ones_row[:], aa[:], start=True, stop=True)
                arep = sb.tile([P, HID], BF, tag="arep", name="arepsb")
                nc.vector.tensor_copy(arep[:], arep_ps[:])
                crep_ps = psum.tile([P, HID], F32, tag="mm", name="crep")
                nc.tensor.matmul(crep_ps[:], ones_row[:], ccs[:], start=True, stop=True)
                crep = sb.tile([P, HID], BF, tag="crep", name="crepsb")
                nc.vector.tensor_copy(crep[:], crep_ps[:])
                ps_pool = psum.tile([G, 1], F32, tag="stat1", name="poolps", bufs=1)
                for t in range(NG):
                    xn = xnew[:, t * HID:(t + 1) * HID]
                    y1 = sb.tile([P, HID], BF, tag="y1", name="y1")
                    nc.vector.tensor_mul(y1[:], xn, arep[:])
                    ybn = sb.tile([P, HID], BF, tag="ybn", name="ybn")
                    nc.vector.tensor_add(ybn[:], y1[:], crep[:])
                    yr = sb.tile([P, HID], BF, tag="yr", name="yr")
                    # leaky relu: max(x, 0.1x)
                    nc.vector.tensor_scalar_mul(y1[:], ybn[:], 0.1)
                    nc.vector.tensor_max(yr[:], ybn[:], y1[:])
                    mc2 = sb.tile([P, 1], F32, tag="mc2", name="mc2")
                    nc.sync.dma_start(out=mc2[:], in_=maskcol[t * P:(t + 1) * P, :])
                    s1 = sb.tile([P, 2], F32, tag="s1", name="s1")
                    nc.vector.tensor_mul(scr_res[:], yr[:], h1r_sb[:])
                    nc.vector.reduce_sum(s1[:, 0:1], scr_res[:], axis=mybir.AxisListType.X)
                    nc.vector.tensor_mul(scr_res[:], yr[:], h2r_sb[:])
                    nc.vector.reduce_sum(s1[:, 1:2], scr_res[:], axis=mybir.AxisListType.X)
                    ic = sb.tile([P, 1], F32, tag="ic", name="ic")
                    nc.sync.dma_start(out=ic[:], in_=invcnt[t * P:(t + 1) * P, :])
                    yv = sb.tile([P, 1], F32, tag="yv", name="yv")
                    nc.vector.tensor_mul(yv[:], s1[:, 0:1], ic[:])
                    yw = sb.tile([P, 1], BF, tag="yw", name="yw")
                    nc.vector.tensor_add(yw[:], yv[:], s1[:, 1:2])
                    # mask pads (bn shifts pads off zero)
                    nc.vector.tensor_scalar_mul(yw[:], yw[:], mc2[:, 0:1])
                    pt_sb = sb.tile([P, G], BF, tag="ptsb", name="ptsb")
                    nc.sync.dma_start(out=pt_sb[:], in_=p_t[t * P:(t + 1) * P, :])
                    nc.tensor.matmul(ps_pool[:], pt_sb[:], yw[:],
                                     start=(t == 0), stop=(t == NG - 1))
                pool_sb = sb.tile([G, 1], F32, tag="poolsb", name="poolsb")
                nc.vector.tensor_copy(pool_sb[:], ps_pool[:])
                nc.sync.dma_start(out=pool_in[:], in_=pool_sb[:])
                nc.gpsimd.collective_compute("AllReduce", mybir.AluOpType.add,
                                             replica_groups=cg,
                                             ins=[pool_in[:].opt()],
                                             outs=[pool_out[:].opt()])
                pr = sb.tile([G, 1], F32, tag="pr", name="pr")
                nc.sync.dma_start(out=pr[:], in_=pool_out[:])
                fin = sb.tile([G, 1], F32, tag="fin", name="fin")
                nc.vector.tensor_scalar_add(fin[:], pr[:], float(head_b))
                nc.sync.dma_start(out=out_ext[:], in_=fin[:])
            scope_out(tok)

        ctx.close()
    return nc


def _bench_pjrt(nc, in_maps, n_cores, repeats):
    """Timing harness: replicate bass2jax.run_bass_via_pjrt's multi-core jit,
    keep inputs device-resident, time batched async executions (slope)."""
    import time as _time

    import jax
    import concourse.mybir as _mybir
    from concourse import bass2jax as b2j
    from jax.experimental.shard_map import shard_map
    from jax.sharding import Mesh, NamedSharding, PartitionSpec

    b2j.install_neuronx_cc_hook()
    partition_name = nc.partition_id_tensor.name if nc.partition_id_tensor else None
    in_names, out_names, out_avals, zero_outs = [], [], [], []
    for alloc in nc.m.functions[0].allocations:
        if not isinstance(alloc, _mybir.MemoryLocationSet):
            continue
        name = alloc.memorylocations[0].name
        if alloc.kind == "ExternalInput":
            if name != partition_name:
                in_names.append(name)
        elif alloc.kind == "ExternalOutput":
            out_names.append(name)
            shape = tuple(alloc.tensor_shape)
            dtype = _mybir.dt.np(alloc.dtype)
            out_avals.append(jax.core.ShapedArray(shape, dtype))
            zero_outs.append(np.zeros(shape, dtype))
    n_params = len(in_names)
    n_outs = len(out_avals)
    all_names = list(in_names) + list(out_names)
    if partition_name is not None:
        all_names.append(partition_name)
    donate = tuple(range(n_params, n_params + n_outs))

    def _body(*args):
        operands = list(args)
        if partition_name is not None:
            operands.append(b2j.partition_id_tensor())
        outs = b2j._bass_exec_p.bind(
            *operands,
            out_avals=tuple(out_avals),
            in_names=tuple(all_names),
            out_names=tuple(out_names),
            lowering_input_output_aliases=(),
            sim_require_finite=True,
            sim_require_nnan=True,
            nc=nc,
        )
        return tuple(outs)

    devices = jax.devices()[:n_cores]
    mesh = Mesh(np.asarray(devices), ("core",))
    in_specs = (PartitionSpec("core"),) * (n_params + n_outs)
    out_specs = (PartitionSpec("core"),) * len(out_names)
    sharded = jax.jit(
        shard_map(_body, mesh=mesh, in_specs=in_specs, out_specs=out_specs,
                  check_rep=False),
        donate_argnums=donate, keep_unused=True)
    sh = NamedSharding(mesh, PartitionSpec("core"))
    concat_in = [
        jax.device_put(
            np.concatenate([np.asarray(in_maps[c][name]) for c in range(n_cores)],
                           axis=0), sh)
        for name in in_names
    ]

    def _zeros():
        return [jax.device_put(np.zeros((n_cores * z.shape[0], *z.shape[1:]),
                                        z.dtype), sh) for z in zero_outs]

    out_arrs = sharded(*concat_in, *_zeros())  # compile + warm
    jax.block_until_ready(out_arrs)

    def run_batch(r):
        zs_list = [_zeros() for _ in range(r)]
        for zs in zs_list:
            jax.block_until_ready(zs)
        t0 = _time.perf_counter()
        outs = [sharded(*concat_in, *zs) for zs in zs_list]
        jax.block_until_ready(outs)
        return _time.perf_counter() - t0

    run_batch(2)  # pipeline warm
    lo, hi = 4, 4 + repeats
    tl = min(run_batch(lo) for _ in range(3))
    th = min(run_batch(hi) for _ in range(3))
    slope_ns = (th - tl) / (hi - lo) * 1e9
    print(f"batch t[{lo}]={tl * 1e3:.2f} ms  t[{hi}]={th * 1e3:.2f} ms")
    print(f"HW exec time: {int(slope_ns)} ns")
    return [
        {name: np.asarray(out_arrs[i]).reshape(n_cores, *out_avals[i].shape)[c]
         for i, name in enumerate(out_names)}
        for c in range(n_cores)
    ]


def kernel(**inputs):
    meta, cores = plan(inputs["edge_index"], inputs["batch"])
    in_maps = build_inmaps(inputs, meta, cores)
    head_b = float(np.asarray(inputs["head_b"]).reshape(-1)[0])
    nc = bacc.Bacc("TRN2")
    build(nc, meta, head_b)
    if not nc.is_finalized():
        nc.finalize()
    reps = int(os.environ.get("KB_BENCH", "0"))
    if reps > 0:
        results = _bench_pjrt(nc, in_maps, NCORES, reps)
        out = np.asarray(results[0]["out"], dtype=np.float32).reshape(G)
        return out
    res = run_bass_kernel_spmd(nc, in_maps, core_ids=list(range(NCORES)))
    out = np.asarray(res.results[0]["out"], dtype=np.float32).reshape(G)
    return out


if __name__ == "__main__":
    import reference
    inputs = {k: np.asarray(v) for k, v in reference.setup_inputs().items()}
    got = kernel(**inputs)
    exp = np.asarray(reference.reference(**inputs))
    rel = np.abs(got - exp).max() / (np.abs(exp).max() + 1e-9)
    print("Relative error:", rel)


# revision 41
# speedup vs baseline: 1.0848x; 1.0848x over previous
"""Trainium2 Bass kernel for AnticipatoryRestaurantGNN (TransformerConv x4 + BN + pool).

Strategy (edge-parallel, dst-sorted, chunked AllGather):
  - Sort edges by dst; partition nodes into 8 contiguous ranges with ~equal
    edge counts. Each core owns its node range and ALL edges pointing into it,
    so segment-softmax and scatter-add are core-local.
  - Per layer, each core computes q/k/v for its own nodes; k/v (bf16) are
    AllGathered in TWO row-chunks (lower/upper half of each core's padded
    node range) so the second AllGather overlaps the first chunk's edge
    sweep (edge phase runs as two sweeps, partial sums merged in node phase).
  - Edge compute per 4-tile (512-edge) window: eps=ea@We and q-row-gather via
    host-baked one-hot matmuls on PE; kv_j = kvg + eps as ONE broadcast DVE
    add; per-head logits via wide mul + reduce; exp on ScalarE; av on DVE;
    scatter-add + denominator accumulate in PSUM via one-hot matmuls.
  - k/v rows are fetched with per-tile indirect DMA. (Batched dma_gather and
    tensor_tensor_reduce both hang this environment's runtime — env-gated
    paths KB_GATHER/KB_TTR exist but default off.) GPSIMD descriptor
    generation (~2us/gather instruction, serialized) is the bottleneck, so
    the per-group node phase (softmax normalize, beta gate, BN partial sums)
    is interleaved into the chunk-1 edge sweep to hide under it; the beta
    x_r.wb term is folded into the skip projection as a 257th matmul column.
  - BatchNorm stats and the final pooled head are AllReduced (tiny).
"""

import math
import os
import sys

sys.path.insert(0, "/opt/trn_rl_repo")

import ml_dtypes
import numpy as np

import concourse.bacc as bacc
import concourse.bass as bass
import concourse.mybir as mybir
import concourse.tile as tile
from concourse import library_config
from concourse.bass_utils import run_bass_kernel_spmd
from concourse.masks import make_identity

BF16 = ml_dtypes.bfloat16

N, E, IN_DIM, EDGE_DIM, HID, L, HEADS, G = 50000, 500000, 64, 16, 256, 4, 4, 64
C = HID // HEADS
NCORES = 8
P = 128
EPS = 1e-5
NCHUNK = 2

F32 = mybir.dt.float32
BF = mybir.dt.bfloat16
I16 = mybir.dt.int16


def _roundup(x, m):
    return (x + m - 1) // m * m


def _wrap16(idx):
    """dma_gather index layout: [16, n//16], idx j -> [j % 16, j // 16]."""
    n = idx.shape[0]
    assert n % 16 == 0
    return np.ascontiguousarray(idx.reshape(n // 16, 16).T)


def plan(edge_index, batch):
    """Host-side layout planning. Returns (meta, per_core_arrays)."""
    src, dst = np.asarray(edge_index[0]), np.asarray(edge_index[1])
    batch = np.asarray(batch)

    order = np.argsort(dst, kind="stable")
    s_src = src[order].astype(np.int64)
    s_dst = dst[order].astype(np.int64)

    deg = np.bincount(dst, minlength=N)
    cum = np.concatenate([[0], np.cumsum(deg)])  # cum[n] = first edge of node n

    # node range split, balanced by edge count, at node boundaries
    ns = [0]
    for i in range(1, NCORES):
        tgt = round(E * i / NCORES)
        ns.append(int(np.searchsorted(cum, tgt, side="left")))
    ns.append(N)
    ns = np.array(ns, dtype=np.int64)
    n_own = np.diff(ns)
    n_pad = _roundup(int(n_own.max()), NCHUNK * P)
    NG = n_pad // P
    NG2 = NG // NCHUNK
    HALF = n_pad // NCHUNK
    BANK = NCORES * HALF
    assert BANK <= 32767

    core_of = np.searchsorted(ns[1:], np.arange(N), side="right")
    loc = np.arange(N) - ns[core_of]
    chunk_of = loc // HALF  # which AG chunk the node's kv row lives in
    bankrow = core_of * HALF + (loc - chunk_of * HALF)

    e_core = core_of[s_dst]  # owning core per sorted edge
    e_chunk = chunk_of[s_src]
    e_bankrow = bankrow[s_src]

    # per (core, group, chunk) edge lists (sorted by src bankrow for locality)
    Tseg = np.zeros((NG, NCHUNK), dtype=np.int64)
    per_core_ed = []
    for c in range(NCORES):
        groups = []
        for g in range(NG):
            lo_node = ns[c] + g * P
            hi_node = min(ns[c] + (g + 1) * P, ns[c + 1])
            if lo_node >= ns[c + 1]:
                eidx = np.arange(0, 0)
            else:
                eidx = np.arange(cum[lo_node], cum[hi_node])
            segs = []
            for r in range(NCHUNK):
                er = eidx[e_chunk[eidx] == r]
                er = er[np.argsort(e_bankrow[er], kind="stable")]
                segs.append(er)
                Tseg[g, r] = max(Tseg[g, r], _roundup(len(er), P) // P)
            groups.append(segs)
        per_core_ed.append(groups)

    TOTE = int(Tseg.sum()) * P  # padded edges per core (same on all cores)

    counts = np.bincount(batch, minlength=G).astype(np.float64)

    meta = dict(ns=ns, n_pad=n_pad, NG=NG, NG2=NG2, HALF=HALF, BANK=BANK,
                Tseg=Tseg, TOTE=TOTE, order=order, counts=counts)

    cores = []
    for c in range(NCORES):
        kv_idx = np.zeros(TOTE, dtype=np.int64)
        a_t2 = np.zeros((P, TOTE), dtype=BF16)   # [dst? no: edge-partition one-hot]
        a_gt2 = np.zeros((P, TOTE), dtype=BF16)
        ea_sel = np.full(TOTE, -1, dtype=np.int64)
        off = 0
        for g in range(NG):
            lo_node = ns[c] + g * P
            for r in range(NCHUNK):
                el = per_core_ed[c][g][r]
                T = int(Tseg[g, r])
                if T == 0:
                    continue
                npad = T * P
                k = len(el)
                kvv = np.full(npad, c * HALF, dtype=np.int64)  # pad: valid row
                kvv[:k] = e_bankrow[el]
                kv_idx[off:off + npad] = kvv
                dr = np.full(npad, -1, dtype=np.int64)
                if k:
                    dr[:k] = s_dst[el] - lo_node
                atb = np.zeros((npad, P), dtype=np.float32)
                valid = dr >= 0
                atb[np.arange(npad)[valid], dr[valid]] = 1.0
                for t in range(T):
                    blk = atb[t * P:(t + 1) * P]  # [128e, 128d]
                    a_t2[:, off + t * P: off + (t + 1) * P] = blk.astype(BF16)
                    a_gt2[:, off + t * P: off + (t + 1) * P] = blk.T.astype(BF16)
                ea_sel[off:off + k] = el
                off += npad
        assert off == TOTE

        kv16 = _wrap16(kv_idx.astype(np.int16))  # [16, TOTE//16]
        TT = TOTE // P
        kv_idx32 = np.ascontiguousarray(kv_idx.reshape(TT, P).T.astype(np.int32))
        kvidx_row = np.ascontiguousarray(kv_idx.astype(np.int32)[None, :])

        nn = int(n_own[c])
        invcnt = np.zeros((n_pad, 1), dtype=np.float32)
        nodes = np.arange(ns[c], ns[c + 1])
        invcnt[:nn, 0] = 1.0 / np.maximum(counts[batch[nodes]], 1.0)
        maskcol = np.zeros((n_pad, 1), dtype=np.float32)
        maskcol[:nn, 0] = 1.0
        p_t = np.zeros((n_pad, G), dtype=np.float32)
        p_t[np.arange(nn), batch[nodes]] = 1.0
        maskrep = np.repeat(maskcol[(NG - 1) * P:NG * P, 0][None, :], P, axis=0)

        cores.append(dict(kv16=kv16, kv_idx32=kv_idx32, kvidx_row=kvidx_row,
                          a_t2=a_t2, a_gt2=a_gt2,
                          ea_sel=ea_sel, invcnt=invcnt, maskcol=maskcol,
                          maskrep=maskrep, p_t=p_t.astype(BF16), nn=nn))
    return meta, cores


def build_inmaps(inputs, meta, cores):
    """Build the per-core in_maps dict for run_bass_kernel_spmd."""
    ns, n_pad = meta["ns"], meta["n_pad"]
    order = meta["order"]
    ea_sorted = np.asarray(inputs["edge_attr"])[order]  # [E, 16] in dst-sorted order

    def f32(x):
        return np.ascontiguousarray(np.asarray(x, dtype=np.float32))

    def bf(x):
        return np.ascontiguousarray(np.asarray(x, dtype=np.float32).astype(BF16))

    nf = np.asarray(inputs["node_features"])
    Wbeta = np.asarray(inputs["Wbeta"])  # [L, 768, 1]
    head_W = np.asarray(inputs["head_W"])  # [512, 1]

    shared = {}
    shared["proj_W"] = bf(inputs["proj_W"])                      # [64,256]
    shared["proj_b"] = bf(inputs["proj_b"][None, :])             # [1,256]
    for l in range(L):
        shared[f"Wq{l}"] = bf(inputs["Wq"][l])
        shared[f"Wk{l}"] = bf(inputs["Wk"][l])
        shared[f"Wv{l}"] = bf(inputs["Wv"][l])
        shared[f"Wsk{l}"] = bf(inputs["Wskip"][l])
        shared[f"bq{l}"] = bf(inputs["bq"][l][None, :])
        shared[f"bk{l}"] = bf((inputs["bk"][l] + inputs["be"][l])[None, :])
        shared[f"bv{l}"] = bf((inputs["bv"][l] + inputs["be"][l])[None, :])
        shared[f"bsk{l}"] = bf(inputs["bskip"][l][None, :])
        shared[f"We{l}"] = bf(inputs["We"][l])                   # [16,256]
        w1, w2, w3 = Wbeta[l, :HID, 0], Wbeta[l, HID:2 * HID, 0], Wbeta[l, 2 * HID:, 0]
        shared[f"wa{l}"] = bf(np.repeat((w1 + w3)[None, :], P, axis=0))   # [128,256]
        shared[f"wb{l}"] = bf(np.repeat((w2 - w3)[None, :], P, axis=0))
        wbv = (w2 - w3).astype(np.float64)
        wsk64 = np.asarray(inputs["Wskip"][l], dtype=np.float64)
        wsbv = wsk64 @ wbv                                   # [256]
        shared[f"wsb{l}"] = bf(np.stack([wsbv[:P], wsbv[P:]], axis=1))  # [128,2]
        bwc = float(np.asarray(inputs["bskip"][l], dtype=np.float64) @ wbv)
        shared[f"bwr{l}"] = f32(np.full((P, 1), bwc))
        shared[f"gam{l}"] = f32(inputs["bn_gamma"][l][None, :])
        shared[f"bet{l}"] = f32(inputs["bn_beta"][l][None, :])
    shared["h1r"] = bf(np.repeat(head_W[:HID, 0][None, :], P, axis=0))
    shared["h2r"] = bf(np.repeat(head_W[HID:, 0][None, :], P, axis=0))

    in_maps = []
    for c in range(NCORES):
        cc = cores[c]
        m = dict(shared)
        nn = cc["nn"]
        nft = np.zeros((IN_DIM, n_pad), dtype=np.float32)
        nft[:, :nn] = nf[ns[c]:ns[c + 1]].T
        m["nf_t"] = nft.astype(BF16)
        ea = np.zeros((meta["TOTE"], EDGE_DIM), dtype=np.float32)
        sel = cc["ea_sel"]
        ea[sel >= 0] = ea_sorted[sel[sel >= 0]]
        m["ea_t"] = np.ascontiguousarray(ea.T.astype(BF16))     # [16, TOTE] bf16
        m["a_t2"] = cc["a_t2"]                                  # [128, TOTE] bf16
        m["a_gt2"] = cc["a_gt2"]
        m["kv16"] = cc["kv16"]                                  # [16, TOTE//16] int16
        m["kv_idx32"] = cc["kv_idx32"]                          # [128, TOTE//128] int32
        m["kvidx_row"] = cc["kvidx_row"]                        # [1, TOTE] int32
        m["invcnt"] = cc["invcnt"]
        m["maskcol"] = cc["maskcol"]
        m["maskrep"] = cc["maskrep"].astype(BF16)
        m["p_t"] = cc["p_t"]
        in_maps.append(m)
    return in_maps


def build(nc, meta, head_b):
    n_pad, NG, NG2, HALF, BANK, Tseg, TOTE = (
        meta["n_pad"], meta["NG"], meta["NG2"], meta["HALF"], meta["BANK"],
        meta["Tseg"], meta["TOTE"])

    # ---- dram parameters ----
    dp = lambda name, shape, dt: nc.declare_dram_parameter(name, list(shape), dt, isOutput=False)
    nf_t = dp("nf_t", (IN_DIM, n_pad), BF)
    ea_t = dp("ea_t", (EDGE_DIM, TOTE), BF)
    a_t2 = dp("a_t2", (P, TOTE), BF)
    a_gt2 = dp("a_gt2", (P, TOTE), BF)
    kv16 = dp("kv16", (16, TOTE // 16), I16)
    kv_idx32 = dp("kv_idx32", (P, TOTE // P), mybir.dt.int32)
    kvidx_row = dp("kvidx_row", (1, TOTE), mybir.dt.int32)
    invcnt = dp("invcnt", (n_pad, 1), F32)
    maskcol = dp("maskcol", (n_pad, 1), F32)
    maskrep = dp("maskrep", (P, P), BF)
    p_t = dp("p_t", (n_pad, G), BF)
    proj_W = dp("proj_W", (IN_DIM, HID), BF)
    proj_b = dp("proj_b", (1, HID), BF)
    Wq, Wk, Wv, Wsk, bq, bk, bv, bsk, We, wa, wb, gam, bet = ({} for _ in range(13))
    for l in range(L):
        Wq[l] = dp(f"Wq{l}", (HID, HID), BF)
        Wk[l] = dp(f"Wk{l}", (HID, HID), BF)
        Wv[l] = dp(f"Wv{l}", (HID, HID), BF)
        Wsk[l] = dp(f"Wsk{l}", (HID, HID), BF)
        bq[l] = dp(f"bq{l}", (1, HID), BF)
        bk[l] = dp(f"bk{l}", (1, HID), BF)
        bv[l] = dp(f"bv{l}", (1, HID), BF)
        bsk[l] = dp(f"bsk{l}", (1, HID), BF)
        We[l] = dp(f"We{l}", (EDGE_DIM, HID), BF)
        wa[l] = dp(f"wa{l}", (P, HID), BF)
        wb[l] = dp(f"wb{l}", (P, HID), BF)
        wa[l, "wsb"] = dp(f"wsb{l}", (P, 2), BF)
        wa[l, "bwr"] = dp(f"bwr{l}", (P, 1), F32)
        gam[l] = dp(f"gam{l}", (1, HID), F32)
        bet[l] = dp(f"bet{l}", (1, HID), F32)
    h1r = dp("h1r", (P, HID), BF)
    h2r = dp("h2r", (P, HID), BF)
    out_ext = nc.declare_dram_parameter("out", [G, 1], F32, isOutput=True)

    cg = [list(range(NCORES))]

    with tile.TileContext(nc) as tc:
        from contextlib import ExitStack
        ctx = ExitStack()

        # ---- DRAM tiles (pool.tile keeps addr_space; tc.tile drops it) ----
        dpool = ctx.enter_context(tc.tile_pool(name="dram", bufs=1, space="DRAM"))
        shared_kw = ({"addr_space": "Shared"}
                     if os.environ.get("KB_SHARED", "0") == "1" else {})
        kv_slab = [[dpool.tile([HALF, 2 * HID], BF, name=f"kv_slab{l}_{r}")
                    for r in range(NCHUNK)] for l in range(L)]
        q_slab = dpool.tile([n_pad, HID], BF, name="q_slab")
        kv_bank = [[dpool.tile([BANK, 2 * HID], BF, name=f"kv_bank{l}_{r}",
                               **shared_kw) for r in range(NCHUNK)]
                   for l in range(L)]
        st_in = dpool.tile([2, HID], F32, name="st_in")
        st_out = [dpool.tile([2, HID], F32, name=f"st_out{l}", **shared_kw)
                  for l in range(L)]
        pool_in = dpool.tile([G, 1], F32, name="pool_in")
        pool_out = dpool.tile([G, 1], F32, name="pool_out", **shared_kw)

        # ---- persistent SBUF ----
        res = ctx.enter_context(tc.tile_pool(name="res", bufs=1))
        x_T = [res.tile([P, n_pad], BF, tag=f"xT{h}", name=f"xT{h}") for h in range(2)]
        xnew = res.tile([P, NG * HID], BF, tag="xnew", name="xnew")
        out_sbA = res.tile([P, NG * HID], BF, tag="outsbA", name="outsbA")
        out_sbB = res.tile([P, NG * HID], BF, tag="outsbB", name="outsbB")
        scr_res = res.tile([P, HID], F32, tag="scrres", name="scrres")
        scr_bf = res.tile([P, HID], BF, tag="scrbf", name="scrbf")
        den_sbA = res.tile([P, NG * HEADS], F32, tag="densbA", name="densbA")
        den_sbB = res.tile([P, NG * HEADS], F32, tag="densbB", name="densbB")
        ident = res.tile([P, P], BF, tag="ident", name="ident")
        make_identity(nc, ident[:])
        ones_row = res.tile([1, P], BF, tag="onesr", name="onesr")
        nc.vector.memset(ones_row[:], 1.0)
        ones_col = res.tile([P, 1], BF, tag="onesc", name="onesc")
        nc.vector.memset(ones_col[:], 1.0)
        one1 = res.tile([1, 1], BF, tag="one1", name="one1")
        nc.vector.memset(one1[:], 1.0)
        wa_sb = res.tile([P, HID], BF, tag="wasb", name="wasb")
        wb_sb = res.tile([P, HID], BF, tag="wbsb", name="wbsb")
        wsb_sb = res.tile([P, 2], BF, tag="wsbsb", name="wsbsb")
        bw_sb = res.tile([P, 1], F32, tag="bwsb", name="bwsb")
        aT_sb = res.tile([P, 2], F32, tag="aTsb", name="aTsb")   # col h = scale for half h
        cT_sb = res.tile([P, 2], F32, tag="cTsb", name="cTsb")
        WW = res.tile([P, 8 * HID], BF, tag="WW", name="WW")  # q0 q1 k0 k1 v0 v1 s0 s1
        We_sb = res.tile([EDGE_DIM, HID], BF, tag="Wesb", name="Wesb")
        bias_sb = res.tile([1, 4 * HID], BF, tag="biassb", name="biassb")  # bq bk bv bsk
        KB_GATHER = int(os.environ.get("KB_GATHER", "0"))
        if KB_GATHER == 3:
            gsem = nc.alloc_semaphore(name="kvgsem")
            gcount = [0]
        KB_TTR = os.environ.get("KB_TTR", "0") == "1"
        if KB_GATHER != 1:
            kvidx32_sb = res.tile([P, TOTE // P], mybir.dt.int32, tag="kvidx32",
                                  name="kvidx32")
            nc.sync.dma_start(out=kvidx32_sb[:], in_=kv_idx32[:])

        h1r_sb = res.tile([P, HID], BF, tag="h1rsb", name="h1rsb")
        nc.sync.dma_start(out=h1r_sb[:], in_=h1r[:])
        h2r_sb = res.tile([P, HID], BF, tag="h2rsb", name="h2rsb")
        nc.sync.dma_start(out=h2r_sb[:], in_=h2r[:])

        for h in range(2):
            nc.vector.memset(x_T[h][:], 0.01)
        nc.vector.memset(xnew[:], 0.01)
        nc.vector.memset(aT_sb[:], 1.0)
        nc.vector.memset(cT_sb[:], 0.0)

        wpool = ctx.enter_context(tc.tile_pool(name="wtmp", bufs=1))
        psum = ctx.enter_context(tc.tile_pool(name="ps", bufs=2, space="PSUM"))
        sb = ctx.enter_context(tc.tile_pool(name="sb", bufs=2))

        def scope_in(name):
            sid, _ = nc.enter_named_scope(name, False)
            return (name, sid)

        def scope_out(tok):
            nc.leave_named_scope(tok[0], tok[1], False)

        def load_layer_weights(l):
            for i, W in enumerate((Wq[l], Wk[l], Wv[l], Wsk[l])):
                for h in range(2):
                    nc.sync.dma_start(out=WW[:, (2 * i + h) * HID:(2 * i + h + 1) * HID],
                                      in_=W[h * P:(h + 1) * P, :])
            nc.sync.dma_start(out=We_sb[:], in_=We[l][:])
            for i, b in enumerate((bq[l], bk[l], bv[l], bsk[l])):
                nc.sync.dma_start(out=bias_sb[:, i * HID:(i + 1) * HID], in_=b[:])
            nc.sync.dma_start(out=wa_sb[:], in_=wa[l][:])
            nc.sync.dma_start(out=wb_sb[:], in_=wb[l][:])
            nc.sync.dma_start(out=wsb_sb[:], in_=wa[l, "wsb"][:])
            nc.sync.dma_start(out=bw_sb[:], in_=wa[l, "bwr"][:])

        def proj_into(t, widx, dst_slab):
            """project x_T node-tile t with weight widx (0..3 -> q,k,v,skip).
            Returns (o, ps): sbuf bf16 [P, HID] tile and the psum (skip proj
            also computes column HID = x . (Wskip @ wb) + bskip . wb, the
            beta-gate x_r term). DMAs to dst_slab if given."""
            wcols = HID + 1 if widx == 3 else HID
            ps = psum.tile([P, HID + 1], F32, tag="mm", name="projps")
            nc.tensor.matmul(ps[:, :HID], x_T[0][:, t * P:(t + 1) * P],
                             WW[:, (2 * widx) * HID:(2 * widx + 1) * HID],
                             start=True, stop=False)
            nc.tensor.matmul(ps[:, :HID], x_T[1][:, t * P:(t + 1) * P],
                             WW[:, (2 * widx + 1) * HID:(2 * widx + 2) * HID],
                             start=False, stop=False)
            nc.tensor.matmul(ps[:, :HID], ones_row[:],
                             bias_sb[:, widx * HID:(widx + 1) * HID],
                             start=False, stop=True)
            if widx == 3:
                nc.tensor.matmul(ps[:, HID:HID + 1], x_T[0][:, t * P:(t + 1) * P],
                                 wsb_sb[:, 0:1], start=True, stop=False)
                nc.tensor.matmul(ps[:, HID:HID + 1], x_T[1][:, t * P:(t + 1) * P],
                                 wsb_sb[:, 1:2], start=False, stop=True)
            o = sb.tile([P, HID], BF, tag="projo", name="projo")
            nc.scalar.activation(o[:], ps[:, :HID], mybir.ActivationFunctionType.Copy)
            if dst_slab is not None:
                slab, row0, c0 = dst_slab
                nc.sync.dma_start(out=slab[row0:row0 + P, c0:c0 + HID], in_=o[:])
            return o, ps

        _agn = [0]

        _agn = [0]

        def emit_ag(l, r):
            sid, _ = nc.enter_named_scope(f"ag{_agn[0]}", False)
            nc.gpsimd.collective_compute("AllGather", mybir.AluOpType.bypass,
                                         replica_groups=cg,
                                         ins=[kv_slab[l][r][:].opt()],
                                         outs=[kv_bank[l][r][:].opt()])
            nc.leave_named_scope(f"ag{_agn[0]}", sid, False)
            _agn[0] += 1

        def qkv_phase(l):
            """q/k/v slabs from x_T; AllGather chunk 0 now, chunk 1 is
            triggered later, interleaved into the chunk-0 edge sweep."""
            for r in range(NCHUNK):
                for t2 in range(NG2):
                    t = r * NG2 + t2
                    proj_into(t, 0, (q_slab, t * P, 0))
                    proj_into(t, 1, (kv_slab[l][r], t2 * P, 0))
                    proj_into(t, 2, (kv_slab[l][r], t2 * P, HID))
                if r == 0:
                    emit_ag(l, 0)

        # edge-tile offsets: seg (g, r) starts at toff[g][r] (in 128-tiles)
        toff = np.zeros((NG, NCHUNK), dtype=np.int64)
        acc = 0
        for g in range(NG):
            for r in range(NCHUNK):
                toff[g, r] = acc
                acc += int(Tseg[g, r])

        KB_SKIPEDGE = os.environ.get("KB_SKIPEDGE", "") == "1"

        def edge_seg(l, g, r, out_sb, den_sb):
            """process edge segment (group g, chunk r) gathering from kv_bank[r]."""
            T = 0 if KB_SKIPEDGE else int(Tseg[g, r])
            og = out_sb[:, g * HID:(g + 1) * HID]
            dg = den_sb[:, g * HEADS:(g + 1) * HEADS]
            if T == 0:
                nc.vector.memset(og, 0.0)
                nc.vector.memset(dg, 0.0)
                return
            t0 = int(toff[g, r])
            GW = 4  # tiles per DVE window
            TMAX = int(Tseg.max())
            kvgs = []
            if KB_GATHER == 4:
                rid = sb.tile([1, TMAX * P], mybir.dt.int32, tag="kvrow",
                              name="kvrow", bufs=2)
                nc.sync.dma_start(out=rid[:, :T * P],
                                  in_=kvidx_row[:, t0 * P:(t0 + T) * P])
            if KB_GATHER == 3:
                kvg = sb.tile([P, TMAX * 2 * HID], BF, tag="kvg3", name="kvg3",
                              bufs=2)
                idx = sb.tile([16, TMAX * 8], I16, tag="kvidx", name="kvidx",
                              bufs=2)
                nc.sync.dma_start(out=idx[:, :T * 8],
                                  in_=kv16[:, t0 * 8:(t0 + T) * 8])
                nc.gpsimd.dma_gather(
                    kvg[:, :T * 2 * HID].rearrange("p (t c) -> p t c", c=2 * HID),
                    kv_bank[l][r][:],
                    idx[:, :T * 8],
                    T * P, T * P, 2 * HID).then_inc(gsem, 16)
                gcount[0] += 1
                nc.vector.wait_ge(gsem, 16 * gcount[0])
                kvgs = [kvg] * ((T + GW - 1) // GW)
            for w0 in ([] if KB_GATHER == 3 else list(range(0, T, GW))):
                tw4 = min(GW, T - w0)
                kvg = sb.tile([P, GW * 2 * HID], BF, tag="kvg", name="kvg", bufs=2)
                if KB_GATHER == 1:
                    idx = sb.tile([16, GW * 8], I16, tag="kvidx", name="kvidx", bufs=2)
                    nc.sync.dma_start(out=idx[:, :tw4 * 8],
                                      in_=kv16[:, (t0 + w0) * 8:(t0 + w0 + tw4) * 8])
                    nc.gpsimd.dma_gather(
                        kvg[:, :tw4 * 2 * HID].rearrange("p (t c) -> p t c", c=2 * HID),
                        kv_bank[l][r][:],
                        idx[:, :tw4 * 8],
                        tw4 * P, tw4 * P, 2 * HID)
                elif KB_GATHER == 2:
                    nc.gpsimd.indirect_dma_start(
                        out=kvg[:, :tw4 * 2 * HID].rearrange("p (t c) -> p t c",
                                                             c=2 * HID),
                        out_offset=None, in_=kv_bank[l][r][:],
                        in_offset=bass.IndirectOffsetOnAxis(
                            ap=kvidx32_sb[:, t0 + w0:t0 + w0 + tw4], axis=0))
                elif KB_GATHER == 4:
                    for ti in range(tw4):
                        s0 = (w0 + ti) * P
                        nc.gpsimd.indirect_dma_start(
                            out=kvg[:, ti * 2 * HID:(ti + 1) * 2 * HID],
                            out_offset=None, in_=kv_bank[l][r][:],
                            in_offset=bass.IndirectOffsetOnAxis(
                                ap=rid[:, s0:s0 + P], axis=0))
                else:
                    for ti in range(tw4):
                        gt2 = t0 + w0 + ti
                        nc.gpsimd.indirect_dma_start(
                            out=kvg[:, ti * 2 * HID:(ti + 1) * 2 * HID],
                            out_offset=None, in_=kv_bank[l][r][:],
                            in_offset=bass.IndirectOffsetOnAxis(
                                ap=kvidx32_sb[:, gt2:gt2 + 1], axis=0))
                kvgs.append(kvg)
            qg_sb = sb.tile([P, HID], BF, tag="qgrp", name="qgrp", bufs=2)
            nc.sync.dma_start(out=qg_sb[:], in_=q_slab[g * P:(g + 1) * P, :])
            ps_out = psum.tile([P, HID], F32, tag="acc", name="accps", bufs=1)
            ps_den = psum.tile([P, 8], F32, tag="den", name="denps", bufs=1)
            for w0 in range(0, T, GW):
                tw4 = min(GW, T - w0)
                gt = t0 + w0
                kvg = kvgs[w0 // GW]
                kvo0 = w0 * 2 * HID if KB_GATHER == 3 else 0
                ea_sb = sb.tile([EDGE_DIM, GW * P], BF, tag="ea", name="ea")
                nc.sync.dma_start(out=ea_sb[:, :tw4 * P],
                                  in_=ea_t[:, gt * P:(gt + tw4) * P])
                at_sb = sb.tile([P, GW * P], BF, tag="at", name="at")
                nc.sync.dma_start(out=at_sb[:, :tw4 * P],
                                  in_=a_t2[:, gt * P:(gt + tw4) * P])
                agt_sb = sb.tile([P, GW * P], BF, tag="agt", name="agt")
                nc.sync.dma_start(out=agt_sb[:, :tw4 * P],
                                  in_=a_gt2[:, gt * P:(gt + tw4) * P])
                eps_sb = sb.tile([P, GW * HID], BF, tag="epssb", name="epssb")
                qi_sb = sb.tile([P, GW * HID], BF, tag="qisb", name="qisb")
                for hh in range(0, tw4, 2):
                    hw = min(2, tw4 - hh)
                    eps_ps = psum.tile([P, 2 * HID], F32, tag="mm", name="epsps")
                    qps = psum.tile([P, 2 * HID], F32, tag="mm2", name="qps")
                    for j in range(hw):
                        t = hh + j
                        nc.tensor.matmul(eps_ps[:, j * HID:(j + 1) * HID],
                                         ea_sb[:, t * P:(t + 1) * P], We_sb[:],
                                         start=True, stop=True)
                        nc.tensor.matmul(qps[:, j * HID:(j + 1) * HID],
                                         agt_sb[:, t * P:(t + 1) * P], qg_sb[:],
                                         start=True, stop=True)
                    nc.scalar.activation(eps_sb[:, hh * HID:(hh + hw) * HID],
                                         eps_ps[:, :hw * HID],
                                         mybir.ActivationFunctionType.Copy)
                    nc.scalar.activation(qi_sb[:, hh * HID:(hh + hw) * HID],
                                         qps[:, :hw * HID],
                                         mybir.ActivationFunctionType.Copy)
                kvj = sb.tile([P, GW * 2 * HID], BF, tag="kvj", name="kvj")
                nc.vector.tensor_tensor(
                    out=kvj[:, :tw4 * 2 * HID
                            ].rearrange("p (t kv c) -> p t kv c", kv=2, c=HID),
                    in0=kvg[:, kvo0:kvo0 + tw4 * 2 * HID
                            ].rearrange("p (t kv c) -> p t kv c", kv=2, c=HID),
                    in1=eps_sb[:, :tw4 * HID
                               ].rearrange("p (t o c) -> p t o c", o=1, c=HID
                                           ).to_broadcast([P, tw4, 2, HID]),
                    op=mybir.AluOpType.add)
                kvjv = kvj[:, :tw4 * 2 * HID].rearrange("p (t k) -> p t k",
                                                        k=2 * HID)
                prod = sb.tile([P, GW * HID], BF, tag="prod", name="prod")
                nc.vector.tensor_tensor(
                    out=prod[:, :tw4 * HID].rearrange("p (t c) -> p t c", c=HID),
                    in0=qi_sb[:, :tw4 * HID].rearrange("p (t c) -> p t c", c=HID),
                    in1=kvjv[:, :, :HID],
                    op=mybir.AluOpType.mult)
                logit = sb.tile([P, GW * HEADS], F32, tag="logit", name="logit")
                nc.vector.reduce_sum(
                    logit[:, :tw4 * HEADS],
                    prod[:, :tw4 * HID].rearrange("p (th c) -> p th c", c=C),
                    axis=mybir.AxisListType.X)
                alpha = sb.tile([P, GW * HEADS], BF, tag="alpha", name="alpha")
                nc.scalar.activation(alpha[:, :tw4 * HEADS], logit[:, :tw4 * HEADS],
                                     mybir.ActivationFunctionType.Exp,
                                     scale=1.0 / math.sqrt(C))
                av = sb.tile([P, GW * HID], BF, tag="av", name="av")
                nc.vector.tensor_tensor(
                    out=av[:, :tw4 * HID
                           ].rearrange("p (t h c) -> p t h c", h=HEADS, c=C),
                    in0=kvjv[:, :, HID:].rearrange("p t (h c) -> p t h c", c=C),
                    in1=alpha[:, :tw4 * HEADS
                              ].rearrange("p (t h o) -> p t h o", h=HEADS, o=1
                                          ).to_broadcast([P, tw4, HEADS, C]),
                    op=mybir.AluOpType.mult)
                first = w0 == 0
                for t in range(tw4):
                    last = w0 + t == T - 1
                    nc.tensor.matmul(ps_out[:], at_sb[:, t * P:(t + 1) * P],
                                     av[:, t * HID:(t + 1) * HID],
                                     start=(first and t == 0), stop=last)
                    nc.tensor.matmul(ps_den[:, :HEADS], at_sb[:, t * P:(t + 1) * P],
                                     alpha[:, t * HEADS:(t + 1) * HEADS],
                                     start=(first and t == 0), stop=last)
            nc.scalar.activation(og, ps_out[:], mybir.ActivationFunctionType.Copy)
            nc.scalar.activation(dg, ps_den[:, :HEADS],
                                 mybir.ActivationFunctionType.Copy)

        # ================= init: x0 = nf @ proj_W + proj_b =================
        tok = scope_in("x0qkv")
        pw_sb = wpool.tile([IN_DIM, HID], BF, tag="pw", name="pw")
        nc.sync.dma_start(out=pw_sb[:], in_=proj_W[:])
        pb_sb = wpool.tile([1, HID], BF, tag="pb", name="pb")
        nc.sync.dma_start(out=pb_sb[:], in_=proj_b[:])
        load_layer_weights(0)
        for r in range(NCHUNK):
            for t2 in range(NG2):
                t = r * NG2 + t2
                nfs = sb.tile([IN_DIM, P], BF, tag="nfs", name="nfs")
                nc.sync.dma_start(out=nfs[:], in_=nf_t[:, t * P:(t + 1) * P])
                ps = psum.tile([P, HID], F32, tag="mm", name="x0ps")
                nc.tensor.matmul(ps[:], nfs[:], pw_sb[:],
                                 start=True, stop=False)
                nc.tensor.matmul(ps[:], ones_row[:], pb_sb[:], start=False, stop=True)
                xr0 = sb.tile([P, HID], BF, tag="xrow0", name="xrow0")
                nc.scalar.activation(xr0[:], ps[:], mybir.ActivationFunctionType.Copy)
                for h in range(2):
                    tp = psum.tile([P, P], BF, tag="mm", name="x0tp")
                    nc.tensor.transpose(tp[:], xr0[:, h * P:(h + 1) * P], ident[:])
                    nc.vector.tensor_copy(x_T[h][:, t * P:(t + 1) * P], tp[:])
                proj_into(t, 0, (q_slab, t * P, 0))
                proj_into(t, 1, (kv_slab[0][r], t2 * P, 0))
                proj_into(t, 2, (kv_slab[0][r], t2 * P, HID))
            if r == 0:
                emit_ag(0, 0)
        scope_out(tok)

        # ================= layers =================
        for l in range(L):
            # ---- edge phase: all chunk-0 segments, then all chunk-1 ----
            tok = scope_in(f"edge{l}")
            ps_sum = psum.tile([1, HID], F32, tag="stat1", name="sumps", bufs=1)
            ps_sq = psum.tile([1, HID], F32, tag="stat2", name="sqps", bufs=1)

            def nodeA_group(t):
                den = sb.tile([P, HEADS], F32, tag="den2", name="den2")
                nc.vector.tensor_add(den[:], den_sbA[:, t * HEADS:(t + 1) * HEADS],
                                     den_sbB[:, t * HEADS:(t + 1) * HEADS])
                dmx = sb.tile([P, HEADS], F32, tag="dmx", name="dmx")
                nc.vector.tensor_scalar_max(dmx[:], den[:], 1e-30)
                rden = sb.tile([P, HEADS], F32, tag="rden", name="rden")
                nc.vector.reciprocal(rden[:], dmx[:])
                outm = sb.tile([P, HID], BF, tag="outm", name="outm")
                nc.vector.tensor_add(outm[:], out_sbA[:, t * HID:(t + 1) * HID],
                                     out_sbB[:, t * HID:(t + 1) * HID])
                outn = sb.tile([P, HID], BF, tag="outn", name="outn")
                nc.vector.tensor_tensor(
                    out=outn[:].rearrange("p (h c) -> p h c", c=C),
                    in0=outm[:].rearrange("p (h c) -> p h c", c=C),
                    in1=rden[:].rearrange("p (h o) -> p h o", o=1
                                          ).to_broadcast([P, HEADS, C]),
                    op=mybir.AluOpType.mult)
                xr, xr_ps = proj_into(t, 3, None)  # x_r = x@Wskip + bskip
                bl = sb.tile([P, 2], F32, tag="bl", name="bl")
                nc.vector.tensor_mul(scr_res[:], outn[:], wa_sb[:])
                nc.vector.reduce_sum(bl[:, 0:1], scr_res[:], axis=mybir.AxisListType.X)
                blsum = sb.tile([P, 1], F32, tag="blsum", name="blsum")
                nc.vector.tensor_add(blsum[:], bl[:, 0:1], xr_ps[:, HID:HID + 1])
                bsig = sb.tile([P, 1], F32, tag="bsig", name="bsig")
                nc.scalar.activation(bsig[:], blsum[:],
                                     mybir.ActivationFunctionType.Sigmoid,
                                     bias=bw_sb[:, 0:1])
                diff = sb.tile([P, HID], BF, tag="diff", name="diff")
                nc.vector.tensor_sub(diff[:], xr[:], outn[:])
                bd = sb.tile([P, HID], BF, tag="bd", name="bd")
                nc.vector.tensor_scalar_mul(bd[:], diff[:], bsig[:, 0:1])
                xn = xnew[:, t * HID:(t + 1) * HID]
                nc.vector.tensor_add(xn, outn[:], bd[:])
                mc = sb.tile([P, 1], F32, tag="mc", name="mc")
                nc.sync.dma_start(out=mc[:], in_=maskcol[t * P:(t + 1) * P, :])
                nc.vector.tensor_scalar_mul(xn, xn, mc[:, 0:1])
                x2 = sb.tile([P, HID], BF, tag="x2", name="x2")
                nc.vector.tensor_mul(x2[:], xn, xn)
                nc.tensor.matmul(ps_sum[:], ones_col[:], xn,
                                 start=(t == 0), stop=(t == NG - 1))
                nc.tensor.matmul(ps_sq[:], ones_col[:], x2[:],
                                 start=(t == 0), stop=(t == NG - 1))

            for g in range(NG):
                edge_seg(l, g, 0, out_sbA, den_sbA)
                if g == 3:
                    emit_ag(l, 1)
            if NG <= 3:
                emit_ag(l, 1)
            for g in range(NG):
                edge_seg(l, g, 1, out_sbB, den_sbB)
                nodeA_group(g)
            scope_out(tok)

            # ---- BN stats all-reduce + finalize ----
            tok = scope_in(f"stats{l}")
            st_a = sb.tile([1, HID], F32, tag="stsa", name="stsa")
            nc.vector.tensor_copy(st_a[:], ps_sum[:])
            st_b = sb.tile([1, HID], F32, tag="stsb", name="stsb")
            nc.vector.tensor_copy(st_b[:], ps_sq[:])
            nc.sync.dma_start(out=st_in[0:1, :], in_=st_a[:])
            nc.sync.dma_start(out=st_in[1:2, :], in_=st_b[:])
            nc.gpsimd.collective_compute("AllReduce", mybir.AluOpType.add,
                                         replica_groups=cg,
                                         ins=[st_in[:].opt()], outs=[st_out[l][:].opt()])
            str_a = sb.tile([1, HID], F32, tag="stra", name="stra")
            nc.sync.dma_start(out=str_a[:], in_=st_out[l][0:1, :])
            str_b = sb.tile([1, HID], F32, tag="strb", name="strb")
            nc.sync.dma_start(out=str_b[:], in_=st_out[l][1:2, :])
            mean = sb.tile([1, HID], F32, tag="mean", name="mean")
            nc.vector.tensor_scalar_mul(mean[:], str_a[:], 1.0 / N)
            var = sb.tile([1, HID], F32, tag="var", name="var")
            nc.vector.tensor_scalar_mul(var[:], str_b[:], 1.0 / N)
            msq = sb.tile([1, HID], F32, tag="msq", name="msq")
            nc.vector.tensor_mul(msq[:], mean[:], mean[:])
            nc.vector.tensor_sub(var[:], var[:], msq[:])
            nc.vector.tensor_scalar_add(var[:], var[:], EPS)
            sd = sb.tile([1, HID], F32, tag="sd", name="sd")
            nc.scalar.activation(sd[:], var[:], mybir.ActivationFunctionType.Sqrt)
            rstd = sb.tile([1, HID], F32, tag="rstd", name="rstd")
            nc.vector.reciprocal(rstd[:], sd[:])
            gam_sb = sb.tile([1, HID], F32, tag="gamsb", name="gamsb")
            nc.sync.dma_start(out=gam_sb[:], in_=gam[l][:])
            bet_sb = sb.tile([1, HID], F32, tag="betsb", name="betsb")
            nc.sync.dma_start(out=bet_sb[:], in_=bet[l][:])
            aa = sb.tile([1, HID], BF, tag="aa", name="aa")
            nc.vector.tensor_mul(aa[:], gam_sb[:], rstd[:])
            ac = sb.tile([1, HID], F32, tag="acs", name="acs")
            nc.vector.tensor_mul(ac[:], mean[:], aa[:])
            ccs = sb.tile([1, HID], BF, tag="ccs", name="ccs")
            nc.vector.tensor_sub(ccs[:], bet_sb[:], ac[:])
            for h in range(2):
                tpa = psum.tile([P, 2], F32, tag="mm", name="tpa")
                nc.tensor.matmul(tpa[:, 0:1], aa[0:1, h * P:(h + 1) * P], one1[:],
                                 start=True, stop=True)
                nc.tensor.matmul(tpa[:, 1:2], ccs[0:1, h * P:(h + 1) * P], one1[:],
                                 start=True, stop=True)
                nc.vector.tensor_copy(aT_sb[:, h:h + 1], tpa[:, 0:1])
                nc.vector.tensor_copy(cT_sb[:, h:h + 1], tpa[:, 1:2])
            scope_out(tok)

            # ---- node phase B: transpose + BN apply + LeakyReLU -> x_T ----
            tok = scope_in(f"nodeB{l}" if l < L - 1 else "pool")
            if l < L - 1:
                mrep = sb.tile([P, P], BF, tag="mrep", name="mrep")
                nc.sync.dma_start(out=mrep[:], in_=maskrep[:])
                for t in range(NG):
                    for h in range(2):
                        tp = psum.tile([P, P], BF, tag="mm", name="xtp")
                        nc.tensor.transpose(
                            tp[:], xnew[:, t * HID + h * P:t * HID + (h + 1) * P],
                            ident[:])
                        dst = x_T[h][:, t * P:(t + 1) * P]
                        nc.scalar.activation(dst, tp[:],
                                             mybir.ActivationFunctionType.Lrelu,
                                             bias=cT_sb[:, h:h + 1],
                                             scale=aT_sb[:, h:h + 1],
                                             alpha=0.1)
                        if t == NG - 1:
                            nc.vector.tensor_mul(dst, dst, mrep[:])
                load_layer_weights(l + 1)
                qkv_phase(l + 1)
            else:
                # ---- pooling + head (x of last layer = BN+lrelu of xnew) ----
                arep_ps = psum.tile([P, HID], F32, tag="mm", name="arep")
                nc.tensor.matmul(arep_ps[:], ones_row[:], aa[:], start=True, stop=True)
                arep = sb.tile([P, HID], BF, tag="arep", name="arepsb")
                nc.vector.tensor_copy(arep[:], arep_ps[:])
                crep_ps = psum.tile([P, HID], F32, tag="mm", name="crep")
                nc.tensor.matmul(crep_ps[:], ones_row[:], ccs[:], start=True, stop=True)
                crep = sb.tile([P, HID], BF, tag="crep", name="crepsb")
                nc.vector.tensor_copy(crep[:], crep_ps[:])
                ps_pool = psum.tile([G, 1], F32, tag="stat1", name="poolps", bufs=1)
                for t in range(NG):
                    xn = xnew[:, t * HID:(t + 1) * HID]
                    y1 = sb.tile([P, HID], BF, tag="y1", name="y1")
                    nc.vector.tensor_mul(y1[:], xn, arep[:])
                    ybn = sb.tile([P, HID], BF, tag="ybn", name="ybn")
                    nc.vector.tensor_add(ybn[:], y1[:], crep[:])
                    yr = sb.tile([P, HID], BF, tag="yr", name="yr")
                    # leaky relu: max(x, 0.1x)
                    nc.vector.tensor_scalar_mul(y1[:], ybn[:], 0.1)
                    nc.vector.tensor_max(yr[:], ybn[:], y1[:])
                    mc2 = sb.tile([P, 1], F32, tag="mc2", name="mc2")
                    nc.sync.dma_start(out=mc2[:], in_=maskcol[t * P:(t + 1) * P, :])
                    s1 = sb.tile([P, 2], F32, tag="s1", name="s1")
                    nc.vector.tensor_mul(scr_res[:], yr[:], h1r_sb[:])
                    nc.vector.reduce_sum(s1[:, 0:1], scr_res[:], axis=mybir.AxisListType.X)
                    nc.vector.tensor_mul(scr_res[:], yr[:], h2r_sb[:])
                    nc.vector.reduce_sum(s1[:, 1:2], scr_res[:], axis=mybir.AxisListType.X)
                    ic = sb.tile([P, 1], F32, tag="ic", name="ic")
                    nc.sync.dma_start(out=ic[:], in_=invcnt[t * P:(t + 1) * P, :])
                    yv = sb.tile([P, 1], F32, tag="yv", name="yv")
                    nc.vector.tensor_mul(yv[:], s1[:, 0:1], ic[:])
                    yw = sb.tile([P, 1], BF, tag="yw", name="yw")
                    nc.vector.tensor_add(yw[:], yv[:], s1[:, 1:2])
                    # mask pads (bn shifts pads off zero)
                    nc.vector.tensor_scalar_mul(yw[:], yw[:], mc2[:, 0:1])
                    pt_sb = sb.tile([P, G], BF, tag="ptsb", name="ptsb")
                    nc.sync.dma_start(out=pt_sb[:], in_=p_t[t * P:(t + 1) * P, :])
                    nc.tensor.matmul(ps_pool[:], pt_sb[:], yw[:],
                                     start=(t == 0), stop=(t == NG - 1))
                pool_sb = sb.tile([G, 1], F32, tag="poolsb", name="poolsb")
                nc.vector.tensor_copy(pool_sb[:], ps_pool[:])
                nc.sync.dma_start(out=pool_in[:], in_=pool_sb[:])
                nc.gpsimd.collective_compute("AllReduce", mybir.AluOpType.add,
                                             replica_groups=cg,
                                             ins=[pool_in[:].opt()],
                                             outs=[pool_out[:].opt()])
                pr = sb.tile([G, 1], F32, tag="pr", name="pr")
                nc.sync.dma_start(out=pr[:], in_=pool_out[:])
                fin = sb.tile([G, 1], F32, tag="fin", name="fin")
                nc.vector.tensor_scalar_add(fin[:], pr[:], float(head_b))
                nc.sync.dma_start(out=out_ext[:], in_=fin[:])
            scope_out(tok)

        ctx.close()
    return nc


def _bench_pjrt(nc, in_maps, n_cores, repeats):
    """Timing harness: replicate bass2jax.run_bass_via_pjrt's multi-core jit,
    keep inputs device-resident, time batched async executions (slope)."""
    import time as _time

    import jax
    import concourse.mybir as _mybir
    from concourse import bass2jax as b2j
    from jax.experimental.shard_map import shard_map
    from jax.sharding import Mesh, NamedSharding, PartitionSpec

    b2j.install_neuronx_cc_hook()
    partition_name = nc.partition_id_tensor.name if nc.partition_id_tensor else None
    in_names, out_names, out_avals, zero_outs = [], [], [], []
    for alloc in nc.m.functions[0].allocations:
        if not isinstance(alloc, _mybir.MemoryLocationSet):
            continue
        name = alloc.memorylocations[0].name
        if alloc.kind == "ExternalInput":
            if name != partition_name:
                in_names.append(name)
        elif alloc.kind == "ExternalOutput":
            out_names.append(name)
            shape = tuple(alloc.tensor_shape)
            dtype = _mybir.dt.np(alloc.dtype)
            out_avals.append(jax.core.ShapedArray(shape, dtype))
            zero_outs.append(np.zeros(shape, dtype))
    n_params = len(in_names)
    n_outs = len(out_avals)
    all_names = list(in_names) + list(out_names)
    if partition_name is not None:
        all_names.append(partition_name)
    donate = tuple(range(n_params, n_params + n_outs))

    def _body(*args):
        operands = list(args)
        if partition_name is not None:
            operands.append(b2j.partition_id_tensor())
        outs = b2j._bass_exec_p.bind(
            *operands,
            out_avals=tuple(out_avals),
            in_names=tuple(all_names),
            out_names=tuple(out_names),
            lowering_input_output_aliases=(),
            sim_require_finite=True,
            sim_require_nnan=True,
            nc=nc,
        )
        return tuple(outs)

    devices = jax.devices()[:n_cores]
    mesh = Mesh(np.asarray(devices), ("core",))
    in_specs = (PartitionSpec("core"),) * (n_params + n_outs)
    out_specs = (PartitionSpec("core"),) * len(out_names)
    sharded = jax.jit(
        shard_map(_body, mesh=mesh, in_specs=in_specs, out_specs=out_specs,
                  check_rep=False),
        donate_argnums=donate, keep_unused=True)
    sh = NamedSharding(mesh, PartitionSpec("core"))
    concat_in = [
        jax.device_put(
            np.concatenate([np.asarray(in_maps[c][name]) for c in range(n_cores)],
                           axis=0), sh)
        for name in in_names
    ]

    def _zeros():
        return [jax.device_put(np.zeros((n_cores * z.shape[0], *z.shape[1:]),
                                        z.dtype), sh) for z in zero_outs]

    out_arrs = sharded(*concat_in, *_zeros())  # compile + warm
    jax.block_until_ready(out_arrs)

    def run_batch(r):
        zs_list = [_zeros() for _ in range(r)]
        for zs in zs_list:
            jax.block_until_ready(zs)
        t0 = _time.perf_counter()
        outs = [sharded(*concat_in, *zs) for zs in zs_list]
        jax.block_until_ready(outs)
        return _time.perf_counter() - t0

    run_batch(2)  # pipeline warm
    lo, hi = 4, 4 + repeats
    tl = min(run_batch(lo) for _ in range(3))
    th = min(run_batch(hi) for _ in range(3))
    slope_ns = (th - tl) / (hi - lo) * 1e9
    print(f"batch t[{lo}]={tl * 1e3:.2f} ms  t[{hi}]={th * 1e3:.2f} ms")
    print(f"HW exec time: {int(slope_ns)} ns")
    return [
        {name: np.asarray(out_arrs[i]).reshape(n_cores, *out_avals[i].shape)[c]
         for i, name in enumerate(out_names)}
        for c in range(n_cores)
    ]


def kernel(**inputs):
    meta, cores = plan(inputs["edge_index"], inputs["batch"])
    in_maps = build_inmaps(inputs, meta, cores)
    head_b = float(np.asarray(inputs["head_b"]).reshape(-1)[0])
    nc = bacc.Bacc("TRN2")
    build(nc, meta, head_b)
    if not nc.is_finalized():
        nc.finalize()
    reps = int(os.environ.get("KB_BENCH", "0"))
    if reps > 0:
        results = _bench_pjrt(nc, in_maps, NCORES, reps)
        out = np.asarray(results[0]["out"], dtype=np.float32).reshape(G)
        return out
    res = run_bass_kernel_spmd(nc, in_maps, core_ids=list(range(NCORES)))
    out = np.asarray(res.results[0]["out"], dtype=np.float32).reshape(G)
    return out


if __name__ == "__main__":
    import reference
    inputs = {k: np.asarray(v) for k, v in reference.setup_inputs().items()}
    got = kernel(**inputs)
    exp = np.asarray(reference.reference(**inputs))
    rel = np.abs(got - exp).max() / (np.abs(exp).max() + 1e-9)
    print("Relative error:", rel)
